# revision 1
# baseline (speedup 1.0000x reference)
"""Multi-head attention (B=4, S=2048, D=1024, H=16, causal) on 8 TRN2 NeuronCores.

Sharding: core c -> (batch b = c//2, head-group hg = c%2 of 8 heads).
Per core: QKV projections for its 8 heads (via on-chip transpose of x),
causal attention in transposed layout (scoresT[t, s]), softmax denominator
via an appended ones-column in the att@V matmul, PE ones-broadcast for the
normalization, then a row-parallel output projection producing a partial
[S, D]. Host sums the two head-group partials per batch and adds the bias.

All matmuls run as float32r (1-pass fp22 multiply, fp32 accumulate).
"""

import sys

import numpy as np

for _p in ("/opt/trn_rl_repo", "/root/.axon_site/_ro/trn_rl_repo"):
    if _p not in sys.path:
        sys.path.append(_p)

import concourse.bass as bass
import concourse.tile as tile
from concourse import mybir
from concourse.bass_utils import run_bass_kernel_spmd

F32 = mybir.dt.float32
F32R = mybir.dt.float32r

B, S, D, H, HD = 4, 2048, 1024, 16, 64
P = 128
NPAIR = 4  # head pairs per core (8 heads)
NS = S // 512  # 4 s-runs of 512
NST = S // P  # 16 s-tiles of 128
NDC = D // P  # 8 d-chunks

_WAIT_EXEMPT = {
    "InstEventSemaphore",
    "InstUnconditionalBranch",
    "InstCall",
    "InstRegisterMove",
}


def fix_extra_waits(nc):
    """TRN2 compute-instruction structs encode at most one semaphore wait.
    After Tile scheduling, move extra waits onto engine NOPs inserted just
    before the over-constrained instruction (same engine, final order)."""
    import copy

    # template InstNoOp per engine (nop() appends to the tail block; pop it)
    templates = {}

    def make_nop(engine):
        if engine not in templates:
            nc.engines[engine].nop()
            tail = nc.m.functions[0].blocks[-1]
            insts = tail.instructions
            templates[engine] = insts.pop()
            tail.instructions = insts
        nop = copy.deepcopy(templates[engine])
        nop.name = nc.get_next_instruction_name()
        return nop

    n_fixed = 0
    for fn in nc.m.functions:
        for blk in fn.blocks:
            out = []
            for inst in blk.instructions:
                si = getattr(inst, "sync_info", None)
                if (
                    type(inst).__name__ not in _WAIT_EXEMPT
                    and si is not None
                    and si.on_wait
                    and len(si.on_wait) > 1
                ):
                    waits = list(si.on_wait)
                    for w in waits[:-1]:
                        nop = make_nop(inst.engine)
                        nop.sync_info = mybir.SyncInfo(on_wait=[w], on_update=[])
                        out.append(nop)
                    si.on_wait = [waits[-1]]
                    n_fixed += 1
                out.append(inst)
            blk.instructions = out
    return n_fixed


def build_nc(reps=1):
    nc = bass.Bass()
    x_d = nc.dram_tensor("x", [S, D], F32, kind="ExternalInput")
    wq_d = nc.dram_tensor("wq", [NPAIR, P, NDC, P], F32, kind="ExternalInput")
    wk_d = nc.dram_tensor("wk", [NPAIR, P, NDC, P], F32, kind="ExternalInput")
    wv_d = nc.dram_tensor("wv", [NPAIR, P, NDC, P], F32, kind="ExternalInput")
    wp_d = nc.dram_tensor("wp", [NPAIR, P, D], F32, kind="ExternalInput")
    ident_d = nc.dram_tensor("ident", [P, P], F32, kind="ExternalInput")
    ones_d = nc.dram_tensor("ones", [P, P], F32, kind="ExternalInput")
    trimask_d = nc.dram_tensor("trimask", [P, P], F32, kind="ExternalInput")
    y_d = nc.dram_tensor("y", [S, D], F32, kind="ExternalOutput")

    import contextlib

    with tile.TileContext(nc) as tc:
        rep_ctx = tc.For_i(0, reps, 1) if reps > 1 else contextlib.nullcontext()
        with rep_ctx, tc.tile_pool(name="consts", bufs=1) as consts:
            ident = consts.tile([P, P], F32R, tag="ident")
            nc.sync.dma_start(ident, ident_d[:, :].bitcast(F32R))
            ones_sb = consts.tile([P, P], F32R, tag="ones")
            nc.sync.dma_start(ones_sb, ones_d[:, :].bitcast(F32R))
            trimask = consts.tile([P, P], F32, tag="trimask")
            nc.sync.dma_start(trimask, trimask_d[:, :])
            zeros = consts.tile([P, 384], F32, tag="zeros")
            nc.gpsimd.memset(zeros, 0.0)
            # x transposed: [d-part, d-chunk, t]
            xT = consts.tile([P, NDC, S], F32R, tag="xT")
            # normalized attention output, transposed: [pair-hk part, pair, s]
            OcatT = consts.tile([P, NPAIR, S], F32R, tag="OcatT")

            # ---- P0: build xT via PE transposes ----
            with (
                tc.tile_pool(name="p0", bufs=3) as p0,
                tc.tile_pool(name="ps0", bufs=2, space="PSUM") as ps0,
            ):
                for st in range(NST):
                    xin = p0.tile([P, D], F32R, tag="xin")
                    nc.sync.dma_start(
                        xin, x_d[st * P : (st + 1) * P, :].bitcast(F32R)
                    )
                    for dc in range(NDC):
                        pt = ps0.tile([P, P], F32R, tag="tr")
                        nc.tensor.transpose(
                            pt, xin[:, dc * P : (dc + 1) * P], ident
                        )
                        nc.vector.tensor_copy(
                            out=xT[:, dc, st * P : (st + 1) * P], in_=pt
                        )

            # ---- P1+P2: per head-pair projections + attention ----
            with (
                tc.tile_pool(name="pw", bufs=1) as pw,
                tc.tile_pool(name="pqk", bufs=2) as pqk,
                tc.tile_pool(name="pvt", bufs=1) as pvt,
                tc.tile_pool(name="pvp", bufs=2) as pvp,
                tc.tile_pool(name="pex", bufs=2) as pex,
                tc.tile_pool(name="psM", bufs=2, space="PSUM") as psM,
                tc.tile_pool(name="psS", bufs=2, space="PSUM") as psS,
                tc.tile_pool(name="psO", bufs=2, space="PSUM") as psO,
            ):
                for p in range(NPAIR):
                    wq_sb = pw.tile([P, NDC, P], F32R, tag="wq")
                    nc.sync.dma_start(wq_sb, wq_d[p].bitcast(F32R))
                    wk_sb = pw.tile([P, NDC, P], F32R, tag="wk")
                    nc.sync.dma_start(wk_sb, wk_d[p].bitcast(F32R))
                    wv_sb = pw.tile([P, NDC, P], F32R, tag="wv")
                    nc.sync.dma_start(wv_sb, wv_d[p].bitcast(F32R))

                    QT = pqk.tile([P, S], F32R, tag="QT")
                    KT = pqk.tile([P, S], F32R, tag="KT")
                    VT = pvt.tile([P, S], F32R, tag="VT")
                    for sc in range(NS):
                        s0 = sc * 512
                        for w_sb, dst in ((wq_sb, QT), (wk_sb, KT), (wv_sb, VT)):
                            ps = psM.tile([P, 512], F32, tag="mm512")
                            for dc in range(NDC):
                                nc.tensor.matmul(
                                    ps,
                                    w_sb[:, dc],
                                    xT[:, dc, s0 : s0 + 512],
                                    start=(dc == 0),
                                    stop=(dc == NDC - 1),
                                )
                            nc.vector.tensor_copy(out=dst[:, s0 : s0 + 512], in_=ps)

                    # V natural layout with ones columns: [t-part, tt, (h0 V|1|h1 V|1)]
                    Vp = pvp.tile([P, NST, 130], F32R, tag="Vp")
                    Vp_r = Vp.rearrange("p t (two ko) -> p t two ko", two=2)
                    nc.sync.dma_start(
                        Vp_r[:, :, :, 64:65],
                        ones_d[:, 0:32]
                        .rearrange("p (t two one) -> p t two one", two=2, one=1)
                        .bitcast(F32R),
                    )
                    for tt in range(NST):
                        ptv = psM.tile([P, 512], F32R, tag="mm512")
                        nc.tensor.transpose(
                            ptv[:, 0:P], VT[:, tt * P : (tt + 1) * P], ident
                        )
                        nc.vector.tensor_copy(
                            out=Vp_r[:, tt, :, 0:64],
                            in_=ptv[:, 0:P].rearrange("p (two k) -> p two k", two=2),
                        )

                    # attention, one head at a time (double-buffered PSUM)
                    for h in (0, 1):
                        for sr in range(NS):
                            s0 = sr * 512
                            n_t = 4 * (sr + 1)
                            po = psO.tile([65, 512], F32, tag="attv", name="attv")
                            for tg in range(n_t // 2):
                                pss = psS.tile([P, 2, 512], F32, tag="s", name="s")
                                for i in (0, 1):
                                    tt = tg * 2 + i
                                    nc.tensor.matmul(
                                        pss[:, i],
                                        KT[64 * h : 64 * h + 64, tt * P : (tt + 1) * P],
                                        QT[64 * h : 64 * h + 64, s0 : s0 + 512],
                                        start=True,
                                        stop=True,
                                    )
                                # causal triangle on diagonal tiles only
                                for i in (0, 1):
                                    tt = tg * 2 + i
                                    j = tt - 4 * sr
                                    if j >= 0:
                                        nc.vector.tensor_tensor(
                                            pss[:, i, P * j : P * (j + 1)],
                                            pss[:, i, P * j : P * (j + 1)],
                                            trimask,
                                            mybir.AluOpType.add,
                                        )
                                et = pex.tile([P, 2, 512], F32R, tag="e", name="e")
                                nc.scalar.activation(
                                    out=et,
                                    in_=pss,
                                    func=mybir.ActivationFunctionType.Exp,
                                    scale=float(HD**-0.5),
                                )
                                # zero fully-masked prefix columns (t > all s in col)
                                for i in (0, 1):
                                    tt = tg * 2 + i
                                    j = tt - 4 * sr
                                    if j >= 1:
                                        nc.gpsimd.tensor_copy(
                                            out=et[:, i, 0 : P * j],
                                            in_=zeros[:, 0 : P * j].bitcast(F32R),
                                        )
                                for i in (0, 1):
                                    tt = tg * 2 + i
                                    nc.tensor.matmul(
                                        po,
                                        Vp[:, tt, 65 * h : 65 * h + 65],
                                        et[:, i, :],
                                        start=(tt == 0),
                                        stop=(tt == n_t - 1),
                                    )
                            dn = pex.tile([P, 512], F32R, tag="dn")
                            nc.vector.tensor_copy(
                                out=dn[64:65, :], in_=po[64:65, :]
                            )
                            pb = psM.tile([P, 512], F32, tag="mm512")
                            nc.tensor.matmul(
                                pb[0:64, :],
                                ones_sb[64:65, 0:64],
                                dn[64:65, :],
                                start=True,
                                stop=True,
                            )
                            rb = pex.tile([64, 512], F32, tag="rb")
                            nc.vector.reciprocal(out=rb, in_=pb[0:64, :])
                            nc.vector.tensor_tensor(
                                OcatT[64 * h : 64 * h + 64, p, s0 : s0 + 512],
                                po[0:64, :],
                                rb,
                                mybir.AluOpType.mult,
                            )

            # ---- P3: output projection (row-parallel partial) ----
            with (
                tc.tile_pool(name="p3", bufs=3) as p3,
                tc.tile_pool(name="p3w", bufs=1) as p3w,
                tc.tile_pool(name="ps3", bufs=4, space="PSUM") as ps3,
            ):
                wp_sb = p3w.tile([P, NPAIR, D], F32R, tag="wp")
                for p in range(NPAIR):
                    nc.sync.dma_start(wp_sb[:, p, :], wp_d[p].bitcast(F32R))
                for st in range(NST):
                    for dc2 in range(2):
                        ps = ps3.tile([P, 512], F32, tag="y")
                        for p in range(NPAIR):
                            nc.tensor.matmul(
                                ps,
                                OcatT[:, p, st * P : (st + 1) * P],
                                wp_sb[:, p, dc2 * 512 : (dc2 + 1) * 512],
                                start=(p == 0),
                                stop=(p == NPAIR - 1),
                            )
                        yt = p3.tile([P, 512], F32, tag="yt")
                        nc.scalar.copy(out=yt, in_=ps)
                        nc.sync.dma_start(
                            y_d[st * P : (st + 1) * P, dc2 * 512 : (dc2 + 1) * 512],
                            yt,
                        )

    fix_extra_waits(nc)
    return nc


_NC = None


def _get_nc():
    global _NC
    if _NC is None:
        _NC = build_nc()
    return _NC


def _prep_core_inputs(x, Wq, Wk, Wv, Wp, core):
    b, hg = core // 2, core % 2
    hsl = slice(hg * 8, hg * 8 + 8)

    def prep_w(W):
        # [8, D, HD] -> [pair, dp, dc, (hip k)]
        a = W[hsl].reshape(NPAIR, 2, NDC, P, HD)
        return np.ascontiguousarray(
            a.transpose(0, 3, 2, 1, 4).reshape(NPAIR, P, NDC, P)
        )

    return {
        "x": np.ascontiguousarray(x[b]),
        "wq": prep_w(Wq),
        "wk": prep_w(Wk),
        "wv": prep_w(Wv),
        "wp": np.ascontiguousarray(
            Wp[hg * 512 : (hg + 1) * 512].reshape(NPAIR, P, D)
        ),
        "ident": np.eye(P, dtype=np.float32),
        "ones": np.ones((P, P), dtype=np.float32),
        "trimask": np.where(
            np.arange(P)[None, :] >= np.arange(P)[:, None], 0.0, -1e30
        ).astype(np.float32),
    }


def kernel(trace=False, **inputs):
    x = np.asarray(inputs["x"], dtype=np.float32)
    Wq = np.asarray(inputs["Wq"], dtype=np.float32)
    Wk = np.asarray(inputs["Wk"], dtype=np.float32)
    Wv = np.asarray(inputs["Wv"], dtype=np.float32)
    Wp = np.asarray(inputs["Wp"], dtype=np.float32)
    bp = np.asarray(inputs["bp"], dtype=np.float32)

    nc = _get_nc()
    in_maps = [_prep_core_inputs(x, Wq, Wk, Wv, Wp, c) for c in range(8)]
    res = run_bass_kernel_spmd(nc, in_maps, core_ids=list(range(8)), trace=trace)

    out = np.empty((B, S, D), dtype=np.float32)
    for b in range(B):
        out[b] = res.results[2 * b]["y"] + res.results[2 * b + 1]["y"] + bp
    if trace:
        return out, res
    return out



# revision 17
# speedup vs baseline: 1.1891x; 1.1891x over previous
"""Multi-head attention (B=4, S=2048, D=1024, H=16, causal) on 8 TRN2 NeuronCores.

Sharding: core c -> (batch b = c//2, head-group hg = c%2 of 8 heads).
Per core: QKV projections for its 8 heads (via on-chip transpose of x),
causal attention in transposed layout (scoresT[t, s]), softmax denominator
via an appended ones-column in the att@V matmul, then a row-parallel output
projection producing a partial [S, D]. Host sums the two head-group
partials per batch and adds the bias.

Attention processes both heads of a pair concurrently: the two K=64 score
matmuls land in distinct PE row-groups (tile_position (0,0)/(64,0)) so they
stream simultaneously, keeping the full array active (HAM stays unthrottled).
Causal masking is a single DVE add of a precomputed [-1e30/triangle] mask
per diagonal tile (covers the fully-masked prefix too). The softmax
normalization takes a fast approx reciprocal of the two [1,512] denominator
rows, then one PE broadcast matmul expands both heads' reciprocals.

All matmuls run as float32r (1-pass fp22 multiply, fp32 accumulate).
"""

import sys

import numpy as np

for _p in ("/opt/trn_rl_repo", "/root/.axon_site/_ro/trn_rl_repo"):
    if _p not in sys.path:
        sys.path.append(_p)

import concourse.bass as bass
import concourse.tile as tile
from concourse import mybir
from concourse.bass_utils import run_bass_kernel_spmd

F32 = mybir.dt.float32
F32R = mybir.dt.float32r

B, S, D, H, HD = 4, 2048, 1024, 16, 64
P = 128
NPAIR = 4  # head pairs per core (8 heads)
NS = S // 512  # 4 s-runs of 512
NST = S // P  # 16 s-tiles of 128
NDC = D // P  # 8 d-chunks

_WAIT_EXEMPT = {
    "InstEventSemaphore",
    "InstUnconditionalBranch",
    "InstCall",
    "InstRegisterMove",
}


def fix_extra_waits(nc):
    """TRN2 compute-instruction structs encode at most one semaphore wait.
    After Tile scheduling, move extra waits onto engine NOPs inserted just
    before the over-constrained instruction (same engine, final order)."""
    import copy

    # template InstNoOp per engine (nop() appends to the tail block; pop it)
    templates = {}

    def make_nop(engine):
        if engine not in templates:
            nc.engines[engine].nop()
            tail = nc.m.functions[0].blocks[-1]
            insts = tail.instructions
            templates[engine] = insts.pop()
            tail.instructions = insts
        nop = copy.deepcopy(templates[engine])
        nop.name = nc.get_next_instruction_name()
        return nop

    n_fixed = 0
    for fn in nc.m.functions:
        for blk in fn.blocks:
            out = []
            for inst in blk.instructions:
                si = getattr(inst, "sync_info", None)
                if (
                    type(inst).__name__ not in _WAIT_EXEMPT
                    and si is not None
                    and si.on_wait
                    and len(si.on_wait) > 1
                ):
                    waits = list(si.on_wait)
                    for w in waits[:-1]:
                        nop = make_nop(inst.engine)
                        nop.sync_info = mybir.SyncInfo(on_wait=[w], on_update=[])
                        out.append(nop)
                    si.on_wait = [waits[-1]]
                    n_fixed += 1
                out.append(inst)
            blk.instructions = out
    return n_fixed


def build_nc(reps=1):
    nc = bass.Bass()
    x_d = nc.dram_tensor("x", [S, D], F32, kind="ExternalInput")
    wq_d = nc.dram_tensor("wq", [NPAIR, P, NDC, P], F32, kind="ExternalInput")
    wk_d = nc.dram_tensor("wk", [NPAIR, P, NDC, P], F32, kind="ExternalInput")
    wv_d = nc.dram_tensor("wv", [NPAIR, P, NDC, P], F32, kind="ExternalInput")
    wp_d = nc.dram_tensor("wp", [NPAIR, P, D], F32, kind="ExternalInput")
    ident_d = nc.dram_tensor("ident", [P, P], F32, kind="ExternalInput")
    ones_d = nc.dram_tensor("ones", [P, P], F32, kind="ExternalInput")
    masks_d = nc.dram_tensor("masks", [NS, P, 512], F32, kind="ExternalInput")
    sel2_d = nc.dram_tensor("sel2", [65, P], F32, kind="ExternalInput")
    y_d = nc.dram_tensor("y", [S, D], F32, kind="ExternalOutput")

    import contextlib

    with tile.TileContext(nc) as tc:
        rep_ctx = tc.For_i(0, reps, 1) if reps > 1 else contextlib.nullcontext()
        with rep_ctx, tc.tile_pool(name="consts", bufs=1) as consts:
            ident = consts.tile([P, P], F32R, tag="ident")
            nc.sync.dma_start(ident, ident_d[:, :].bitcast(F32R))
            sel2 = consts.tile([65, P], F32R, tag="sel2")
            nc.sync.dma_start(sel2, sel2_d[:, :].bitcast(F32R))
            # denominator staging rows 0/64; rows 1-63 stay 1.0 (never
            # written) so ln->exp of them is finite where sel2 rows are zero
            dnP = consts.tile([65, S], F32, tag="dnP")
            nc.gpsimd.memset(dnP, 1.0)
            masks = consts.tile([P, NS, 512], F32, tag="masks")
            for j in range(NS):
                nc.sync.dma_start(masks[:, j, :], masks_d[j])
            # x transposed: [d-part, d-chunk, t]
            xT = consts.tile([P, NDC, S], F32R, tag="xT")
            # normalized attention output, transposed: [pair-hk part, pair, s]
            OcatT = consts.tile([P, NPAIR, S], F32R, tag="OcatT")

            # ---- P0: build xT via PE transposes ----
            with (
                tc.tile_pool(name="p0", bufs=3) as p0,
                tc.tile_pool(name="ps0", bufs=2, space="PSUM") as ps0,
            ):
                for st in range(NST):
                    xin = p0.tile([P, D], F32R, tag="xin")
                    nc.sync.dma_start(
                        xin, x_d[st * P : (st + 1) * P, :].bitcast(F32R)
                    )
                    for dc in range(NDC):
                        pt = ps0.tile([P, P], F32R, tag="tr")
                        nc.tensor.transpose(
                            pt, xin[:, dc * P : (dc + 1) * P], ident
                        )
                        nc.vector.tensor_copy(
                            out=xT[:, dc, st * P : (st + 1) * P], in_=pt
                        )

            # ---- P1+P2: per head-pair projections + attention ----
            with (
                tc.tile_pool(name="pw", bufs=1) as pw,
                tc.tile_pool(name="pqk", bufs=2) as pqk,
                tc.tile_pool(name="pvt", bufs=2) as pvt,
                tc.tile_pool(name="pvp", bufs=2) as pvp,
                tc.tile_pool(name="pex", bufs=2) as pex,
                tc.tile_pool(name="pnrm", bufs=1) as pnrm,
                tc.tile_pool(name="psM", bufs=2, space="PSUM") as psM,
                tc.tile_pool(name="psS", bufs=2, space="PSUM") as psS,
                tc.tile_pool(name="psO", bufs=1, space="PSUM") as psO,
            ):
                for p in range(NPAIR):
                    wq_sb = pw.tile([P, NDC, P], F32R, tag="wq")
                    nc.sync.dma_start(wq_sb, wq_d[p].bitcast(F32R))
                    wk_sb = pw.tile([P, NDC, P], F32R, tag="wk")
                    nc.sync.dma_start(wk_sb, wk_d[p].bitcast(F32R))
                    wv_sb = pw.tile([P, NDC, P], F32R, tag="wv")
                    nc.sync.dma_start(wv_sb, wv_d[p].bitcast(F32R))

                    QT = pqk.tile([P, S], F32R, tag="QT")
                    KT = pqk.tile([P, S], F32R, tag="KT")
                    for sc in range(NS):
                        s0 = sc * 512
                        for w_sb, dst in ((wq_sb, QT), (wk_sb, KT)):
                            ps = psM.tile([P, 512], F32, tag="mm512")
                            for dc in range(NDC):
                                nc.tensor.matmul(
                                    ps,
                                    w_sb[:, dc],
                                    xT[:, dc, s0 : s0 + 512],
                                    start=(dc == 0),
                                    stop=(dc == NDC - 1),
                                )
                            nc.vector.tensor_copy(out=dst[:, s0 : s0 + 512], in_=ps)

                    # V natural layout with ones columns:
                    # [t-part, tt, (h0 V|1|h1 V|1)], via VT then PE transpose
                    Vp = pvp.tile([P, NST, 130], F32R, tag="Vp")
                    Vp_r = Vp.rearrange("p t (two ko) -> p t two ko", two=2)
                    nc.sync.dma_start(
                        Vp_r[:, :, :, 64:65],
                        ones_d[:, 0:32]
                        .rearrange("p (t two one) -> p t two one", two=2, one=1)
                        .bitcast(F32R),
                    )
                    for sc in range(NS):
                        s0 = sc * 512
                        VT = pvt.tile([P, 512], F32R, tag="VT")
                        ps = psM.tile([P, 512], F32, tag="mm512")
                        for dc in range(NDC):
                            nc.tensor.matmul(
                                ps,
                                wv_sb[:, dc],
                                xT[:, dc, s0 : s0 + 512],
                                start=(dc == 0),
                                stop=(dc == NDC - 1),
                            )
                        nc.vector.tensor_copy(out=VT, in_=ps)
                        for tq in range(4):
                            tt = sc * 4 + tq
                            ptv = psM.tile([P, 512], F32R, tag="mm512")
                            nc.tensor.transpose(
                                ptv[:, 0:P], VT[:, tq * P : (tq + 1) * P], ident
                            )
                            nc.vector.tensor_copy(
                                out=Vp_r[:, tt, :, 0:64],
                                in_=ptv[:, 0:P].rearrange(
                                    "p (two k) -> p two k", two=2
                                ),
                            )

                    # attention: both heads per t-tile, row-tiled scores
                    for sr in range(NS):
                        s0 = sr * 512
                        n_t = 4 * (sr + 1)
                        po0 = psO.tile([65, 512], F32, tag="po0", name="po0")
                        po1 = psO.tile([65, 512], F32, tag="po1", name="po1")
                        ets = {}

                        def emit_attv(tt):
                            et = ets.pop(tt)
                            nc.tensor.matmul(
                                po0,
                                Vp[:, tt, 0:65],
                                et[:, 0],
                                start=(tt == 0),
                                stop=(tt == n_t - 1),
                            )
                            nc.tensor.matmul(
                                po1,
                                Vp[:, tt, 65:130],
                                et[:, 1],
                                start=(tt == 0),
                                stop=(tt == n_t - 1),
                            )

                        for tt in range(n_t):
                            pss = psS.tile([P, 2, 512], F32, tag="s", name="s")
                            for i in (0, 1):
                                nc.tensor.matmul(
                                    pss[:, i],
                                    KT[64 * i : 64 * i + 64, tt * P : (tt + 1) * P],
                                    QT[64 * i : 64 * i + 64, s0 : s0 + 512],
                                    start=True,
                                    stop=True,
                                )
                            j = tt - 4 * sr
                            if j >= 0:
                                w = P * (j + 1)
                                for i in (0, 1):
                                    nc.vector.tensor_tensor(
                                        pss[:, i, 0:w],
                                        pss[:, i, 0:w],
                                        masks[:, j, 0:w],
                                        mybir.AluOpType.add,
                                    )
                            et = pex.tile([P, 2, 512], F32R, tag="e", name="e")
                            nc.scalar.activation(
                                out=et,
                                in_=pss,
                                func=mybir.ActivationFunctionType.Exp,
                                scale=float(HD**-0.5),
                            )
                            ets[tt] = et
                            if tt >= 1:
                                emit_attv(tt - 1)
                        emit_attv(n_t - 1)

                        # normalization: recip of the two denominator rows,
                        # one broadcast matmul expands both heads
                        # raw attention outputs; normalization is applied
                        # in place before the output projection reads them
                        nc.vector.tensor_copy(
                            out=OcatT[0:64, p, s0 : s0 + 512], in_=po0[0:64, :]
                        )
                        nc.vector.tensor_copy(
                            out=dnP[0:1, s0 : s0 + 512], in_=po0[64:65, :]
                        )
                        nc.vector.tensor_copy(
                            out=OcatT[64:128, p, s0 : s0 + 512], in_=po1[0:64, :]
                        )
                        nc.vector.tensor_copy(
                            out=dnP[64:65, s0 : s0 + 512], in_=po1[64:65, :]
                        )
                        # 1/den = exp(-ln(den)) on the scalar engine (the DVE
                        # reciprocal is ~6.5 ns/elem; ACT is ~1.1)
                        lnt = pnrm.tile([65, 512], F32, tag="lnt")
                        nc.scalar.activation(
                            out=lnt,
                            in_=dnP[0:65, s0 : s0 + 512],
                            func=mybir.ActivationFunctionType.Ln,
                        )
                        rcr = pnrm.tile([65, 512], F32R, tag="rcr")
                        nc.scalar.activation(
                            out=rcr,
                            in_=lnt,
                            func=mybir.ActivationFunctionType.Exp,
                            scale=-1.0,
                        )
                        rb = psM.tile([P, 512], F32, tag="mm512")
                        nc.tensor.matmul(rb, sel2, rcr, start=True, stop=True)
                        rbs = pnrm.tile([P, 512], F32, tag="rbs")
                        nc.vector.tensor_copy(out=rbs, in_=rb)
                        nc.gpsimd.tensor_tensor(
                            OcatT[0:64, p, s0 : s0 + 512],
                            OcatT[0:64, p, s0 : s0 + 512],
                            rbs[0:64, :],
                            mybir.AluOpType.mult,
                        )
                        nc.gpsimd.tensor_tensor(
                            OcatT[64:128, p, s0 : s0 + 512],
                            OcatT[64:128, p, s0 : s0 + 512],
                            rbs[64:128, :],
                            mybir.AluOpType.mult,
                        )

            # ---- P3: output projection (row-parallel partial) ----
            with (
                tc.tile_pool(name="p3", bufs=3) as p3,
                tc.tile_pool(name="p3w", bufs=1) as p3w,
                tc.tile_pool(name="ps3", bufs=4, space="PSUM") as ps3,
            ):
                wp_sb = p3w.tile([P, NPAIR, D], F32R, tag="wp")
                for p in range(NPAIR):
                    nc.sync.dma_start(wp_sb[:, p, :], wp_d[p].bitcast(F32R))
                for st in range(NST):
                    for dc2 in range(2):
                        ps = ps3.tile([P, 512], F32, tag="y")
                        for p in range(NPAIR):
                            nc.tensor.matmul(
                                ps,
                                OcatT[:, p, st * P : (st + 1) * P],
                                wp_sb[:, p, dc2 * 512 : (dc2 + 1) * 512],
                                start=(p == 0),
                                stop=(p == NPAIR - 1),
                            )
                        yt = p3.tile([P, 512], F32, tag="yt")
                        nc.scalar.copy(out=yt, in_=ps)
                        nc.sync.dma_start(
                            y_d[st * P : (st + 1) * P, dc2 * 512 : (dc2 + 1) * 512],
                            yt,
                        )

    fix_extra_waits(nc)
    return nc


_NC = None


def _get_nc():
    global _NC
    if _NC is None:
        _NC = build_nc()
    return _NC


def _prep_core_inputs(x, Wq, Wk, Wv, Wp, core):
    b, hg = core // 2, core % 2
    hsl = slice(hg * 8, hg * 8 + 8)

    def prep_w(W):
        # [8, D, HD] -> [pair, dp, dc, (hip k)]
        a = W[hsl].reshape(NPAIR, 2, NDC, P, HD)
        return np.ascontiguousarray(
            a.transpose(0, 3, 2, 1, 4).reshape(NPAIR, P, NDC, P)
        )

    r = np.arange(P)[:, None]
    c = np.arange(512)[None, :]
    masks = np.stack(
        [np.where(c - P * j >= r, 0.0, -1e30) for j in range(NS)]
    ).astype(np.float32)
    sel2 = np.zeros((65, P), dtype=np.float32)
    sel2[0, 0:64] = 1.0
    sel2[64, 64:128] = 1.0

    return {
        "x": np.ascontiguousarray(x[b]),
        "wq": prep_w(Wq),
        "wk": prep_w(Wk),
        "wv": prep_w(Wv),
        "wp": np.ascontiguousarray(
            Wp[hg * 512 : (hg + 1) * 512].reshape(NPAIR, P, D)
        ),
        "ident": np.eye(P, dtype=np.float32),
        "ones": np.ones((P, P), dtype=np.float32),
        "masks": masks,
        "sel2": sel2,
    }


def kernel(trace=False, **inputs):
    x = np.asarray(inputs["x"], dtype=np.float32)
    Wq = np.asarray(inputs["Wq"], dtype=np.float32)
    Wk = np.asarray(inputs["Wk"], dtype=np.float32)
    Wv = np.asarray(inputs["Wv"], dtype=np.float32)
    Wp = np.asarray(inputs["Wp"], dtype=np.float32)
    bp = np.asarray(inputs["bp"], dtype=np.float32)

    nc = _get_nc()
    in_maps = [_prep_core_inputs(x, Wq, Wk, Wv, Wp, c) for c in range(8)]
    res = run_bass_kernel_spmd(nc, in_maps, core_ids=list(range(8)), trace=trace)

    out = np.empty((B, S, D), dtype=np.float32)
    for b in range(B):
        out[b] = res.results[2 * b]["y"] + res.results[2 * b + 1]["y"] + bp
    if trace:
        return out, res
    return out


# revision 19
# speedup vs baseline: 1.3708x; 1.1528x over previous
"""Multi-head attention (B=4, S=2048, D=1024, H=16, causal) on 8 TRN2 NeuronCores.

Sharding: core c -> (batch b = c//2, head-group hg = c%2 of 8 heads).
Per core: QKV projections for its 8 heads (via on-chip transpose of x),
causal attention in transposed layout (scoresT[t, s]), softmax denominator
via an appended ones-column in the att@V matmul, then a row-parallel output
projection producing a partial [S, D]. Host sums the two head-group
partials per batch and adds the bias.

All matmul operands are bf16 (fp32 accumulation in PSUM). The att@V
stationary operand is padded to the full 128 columns (output rows 65-127
are never read) so every attention matmul exercises the whole PE array and
the HAM clock gate stays at full speed. The two heads of a pair are
processed together: score matmuls for head0/head1 are emitted adjacently
into distinct PE row-groups, one exp covers both heads' score tiles, and
causal masking is one DVE add of a precomputed [-1e30/triangle] mask per
diagonal tile. Softmax normalization computes 1/den as exp(-ln(den)) on
the scalar engine, expands it with one small broadcast matmul, and applies
it in place on the GpSimd engine after raw outputs are copied out.
"""

import sys

import numpy as np

for _p in ("/opt/trn_rl_repo", "/root/.axon_site/_ro/trn_rl_repo"):
    if _p not in sys.path:
        sys.path.append(_p)

import ml_dtypes

import concourse.bass as bass
import concourse.tile as tile
from concourse import mybir
from concourse.bass_utils import run_bass_kernel_spmd

F32 = mybir.dt.float32
BF16 = mybir.dt.bfloat16
NPBF = ml_dtypes.bfloat16

B, S, D, H, HD = 4, 2048, 1024, 16, 64
P = 128
NPAIR = 4  # head pairs per core (8 heads)
NS = S // 512  # 4 s-runs of 512
NST = S // P  # 16 s-tiles of 128
NDC = D // P  # 8 d-chunks

_WAIT_EXEMPT = {
    "InstEventSemaphore",
    "InstUnconditionalBranch",
    "InstCall",
    "InstRegisterMove",
}


def fix_extra_waits(nc):
    """TRN2 compute-instruction structs encode at most one semaphore wait.
    After Tile scheduling, move extra waits onto engine NOPs inserted just
    before the over-constrained instruction (same engine, final order)."""
    import copy

    # template InstNoOp per engine (nop() appends to the tail block; pop it)
    templates = {}

    def make_nop(engine):
        if engine not in templates:
            nc.engines[engine].nop()
            tail = nc.m.functions[0].blocks[-1]
            insts = tail.instructions
            templates[engine] = insts.pop()
            tail.instructions = insts
        nop = copy.deepcopy(templates[engine])
        nop.name = nc.get_next_instruction_name()
        return nop

    n_fixed = 0
    for fn in nc.m.functions:
        for blk in fn.blocks:
            out = []
            for inst in blk.instructions:
                si = getattr(inst, "sync_info", None)
                if (
                    type(inst).__name__ not in _WAIT_EXEMPT
                    and si is not None
                    and si.on_wait
                    and len(si.on_wait) > 1
                ):
                    waits = list(si.on_wait)
                    for w in waits[:-1]:
                        nop = make_nop(inst.engine)
                        nop.sync_info = mybir.SyncInfo(on_wait=[w], on_update=[])
                        out.append(nop)
                    si.on_wait = [waits[-1]]
                    n_fixed += 1
                out.append(inst)
            blk.instructions = out
    return n_fixed


def build_nc(reps=1):
    nc = bass.Bass()
    x_d = nc.dram_tensor("x", [S, D], BF16, kind="ExternalInput")
    wq_d = nc.dram_tensor("wq", [NPAIR, P, NDC, P], BF16, kind="ExternalInput")
    wk_d = nc.dram_tensor("wk", [NPAIR, P, NDC, P], BF16, kind="ExternalInput")
    wv_d = nc.dram_tensor("wv", [NPAIR, P, NDC, P], BF16, kind="ExternalInput")
    wp_d = nc.dram_tensor("wp", [NPAIR, P, D], BF16, kind="ExternalInput")
    ident_d = nc.dram_tensor("ident", [P, P], BF16, kind="ExternalInput")
    ones_d = nc.dram_tensor("ones", [P, P], BF16, kind="ExternalInput")
    masks_d = nc.dram_tensor("masks", [NS, P, 512], F32, kind="ExternalInput")
    sel2_d = nc.dram_tensor("sel2", [65, P], BF16, kind="ExternalInput")
    y_d = nc.dram_tensor("y", [S, D], F32, kind="ExternalOutput")

    import contextlib

    with tile.TileContext(nc) as tc:
        rep_ctx = tc.For_i(0, reps, 1) if reps > 1 else contextlib.nullcontext()
        with rep_ctx, tc.tile_pool(name="consts", bufs=1) as consts:
            ident = consts.tile([P, P], BF16, tag="ident")
            nc.sync.dma_start(ident, ident_d[:, :])
            sel2 = consts.tile([65, P], BF16, tag="sel2")
            nc.sync.dma_start(sel2, sel2_d[:, :])
            # denominator staging rows 0/64; rows 1-63 stay 1.0 (never
            # written) so ln->exp of them is finite where sel2 rows are zero
            dnP = consts.tile([65, S], F32, tag="dnP")
            nc.gpsimd.memset(dnP, 1.0)
            masks = consts.tile([P, NS, 512], F32, tag="masks")
            for j in range(NS):
                nc.sync.dma_start(masks[:, j, :], masks_d[j])
            # x transposed: [d-part, d-chunk, t]
            xT = consts.tile([P, NDC, S], BF16, tag="xT")
            # attention output (raw, then normalized in place), transposed:
            # [pair-hk part, pair, s]
            OcatT = consts.tile([P, NPAIR, S], BF16, tag="OcatT")

            # ---- P0: build xT via PE transposes ----
            with (
                tc.tile_pool(name="p0", bufs=3) as p0,
                tc.tile_pool(name="ps0", bufs=2, space="PSUM") as ps0,
            ):
                for st in range(NST):
                    xin = p0.tile([P, D], BF16, tag="xin")
                    nc.sync.dma_start(xin, x_d[st * P : (st + 1) * P, :])
                    for dc in range(NDC):
                        pt = ps0.tile([P, P], BF16, tag="tr")
                        nc.tensor.transpose(
                            pt, xin[:, dc * P : (dc + 1) * P], ident
                        )
                        nc.vector.tensor_copy(
                            out=xT[:, dc, st * P : (st + 1) * P], in_=pt
                        )

            # ---- P1+P2: per head-pair projections + attention ----
            with (
                tc.tile_pool(name="pw", bufs=2) as pw,
                tc.tile_pool(name="pqk", bufs=2) as pqk,
                tc.tile_pool(name="pvt", bufs=2) as pvt,
                tc.tile_pool(name="pvp", bufs=2) as pvp,
                tc.tile_pool(name="pex", bufs=2) as pex,
                tc.tile_pool(name="pnrm", bufs=1) as pnrm,
                tc.tile_pool(name="psM", bufs=2, space="PSUM") as psM,
                tc.tile_pool(name="psS", bufs=2, space="PSUM") as psS,
                tc.tile_pool(name="psO", bufs=1, space="PSUM") as psO,
            ):
                for p in range(NPAIR):
                    wq_sb = pw.tile([P, NDC, P], BF16, tag="wq")
                    nc.sync.dma_start(wq_sb, wq_d[p])
                    wk_sb = pw.tile([P, NDC, P], BF16, tag="wk")
                    nc.sync.dma_start(wk_sb, wk_d[p])
                    wv_sb = pw.tile([P, NDC, P], BF16, tag="wv")
                    nc.sync.dma_start(wv_sb, wv_d[p])

                    QT = pqk.tile([P, S], BF16, tag="QT")
                    KT = pqk.tile([P, S], BF16, tag="KT")
                    for sc in range(NS):
                        s0 = sc * 512
                        for w_sb, dst in ((wq_sb, QT), (wk_sb, KT)):
                            ps = psM.tile([P, 512], F32, tag="mm512")
                            for dc in range(NDC):
                                nc.tensor.matmul(
                                    ps,
                                    w_sb[:, dc],
                                    xT[:, dc, s0 : s0 + 512],
                                    start=(dc == 0),
                                    stop=(dc == NDC - 1),
                                )
                            nc.vector.tensor_copy(out=dst[:, s0 : s0 + 512], in_=ps)

                    # V natural layout, padded to the full 128 weight columns
                    # per head: [t-part, tt, head, (64 V | 1 ones | 63 junk)].
                    # Matmul output rows 65-127 are never read, so the pad
                    # columns need no initialization.
                    Vp = pvp.tile([P, NST, 2, P], BF16, tag="Vp")
                    nc.sync.dma_start(
                        Vp[:, :, :, 64:65],
                        ones_d[:, 0:32].rearrange(
                            "p (t two one) -> p t two one", two=2, one=1
                        ),
                    )
                    for sc in range(NS):
                        s0 = sc * 512
                        VT = pvt.tile([P, 512], BF16, tag="VT")
                        ps = psM.tile([P, 512], F32, tag="mm512")
                        for dc in range(NDC):
                            nc.tensor.matmul(
                                ps,
                                wv_sb[:, dc],
                                xT[:, dc, s0 : s0 + 512],
                                start=(dc == 0),
                                stop=(dc == NDC - 1),
                            )
                        nc.vector.tensor_copy(out=VT, in_=ps)
                        for tq in range(4):
                            tt = sc * 4 + tq
                            ptv = psM.tile([P, 1024], BF16, tag="mm512")
                            nc.tensor.transpose(
                                ptv[:, 0:P], VT[:, tq * P : (tq + 1) * P], ident
                            )
                            for h in (0, 1):
                                nc.vector.tensor_copy(
                                    out=Vp[:, tt, h, 0:64],
                                    in_=ptv[:, 64 * h : 64 * h + 64],
                                )

                    # attention: both heads per t-tile, row-tiled scores
                    for sr in range(NS):
                        s0 = sr * 512
                        n_t = 4 * (sr + 1)
                        po0 = psO.tile([P, 512], F32, tag="po0", name="po0")
                        po1 = psO.tile([P, 512], F32, tag="po1", name="po1")
                        ets = {}

                        def emit_attv(tt):
                            et = ets.pop(tt)
                            nc.tensor.matmul(
                                po0,
                                Vp[:, tt, 0],
                                et[:, 0],
                                start=(tt == 0),
                                stop=(tt == n_t - 1),
                            )
                            nc.tensor.matmul(
                                po1,
                                Vp[:, tt, 1],
                                et[:, 1],
                                start=(tt == 0),
                                stop=(tt == n_t - 1),
                            )

                        for tt in range(n_t):
                            pss = psS.tile([P, 2, 512], F32, tag="s", name="s")
                            for i in (0, 1):
                                nc.tensor.matmul(
                                    pss[:, i],
                                    KT[64 * i : 64 * i + 64, tt * P : (tt + 1) * P],
                                    QT[64 * i : 64 * i + 64, s0 : s0 + 512],
                                    start=True,
                                    stop=True,
                                )
                            j = tt - 4 * sr
                            if j >= 0:
                                w = P * (j + 1)
                                for i in (0, 1):
                                    nc.vector.tensor_tensor(
                                        pss[:, i, 0:w],
                                        pss[:, i, 0:w],
                                        masks[:, j, 0:w],
                                        mybir.AluOpType.add,
                                    )
                            et = pex.tile([P, 2, 512], BF16, tag="e", name="e")
                            nc.scalar.activation(
                                out=et,
                                in_=pss,
                                func=mybir.ActivationFunctionType.Exp,
                                scale=float(HD**-0.5),
                            )
                            ets[tt] = et
                            if tt >= 1:
                                emit_attv(tt - 1)
                        emit_attv(n_t - 1)

                        # raw attention outputs; normalization is applied
                        # in place before the output projection reads them
                        nc.vector.tensor_copy(
                            out=OcatT[0:64, p, s0 : s0 + 512], in_=po0[0:64, :]
                        )
                        nc.vector.tensor_copy(
                            out=dnP[0:1, s0 : s0 + 512], in_=po0[64:65, :]
                        )
                        nc.vector.tensor_copy(
                            out=OcatT[64:128, p, s0 : s0 + 512], in_=po1[0:64, :]
                        )
                        nc.vector.tensor_copy(
                            out=dnP[64:65, s0 : s0 + 512], in_=po1[64:65, :]
                        )
                        # 1/den = exp(-ln(den)) on the scalar engine (the DVE
                        # reciprocal is ~6.5 ns/elem; ACT is ~1.1)
                        lnt = pnrm.tile([65, 512], F32, tag="lnt")
                        nc.scalar.activation(
                            out=lnt,
                            in_=dnP[0:65, s0 : s0 + 512],
                            func=mybir.ActivationFunctionType.Ln,
                        )
                        rcr = pnrm.tile([65, 512], BF16, tag="rcr")
                        nc.scalar.activation(
                            out=rcr,
                            in_=lnt,
                            func=mybir.ActivationFunctionType.Exp,
                            scale=-1.0,
                        )
                        rb = psM.tile([P, 512], F32, tag="mm512")
                        nc.tensor.matmul(rb, sel2, rcr, start=True, stop=True)
                        rbs = pnrm.tile([P, 512], BF16, tag="rbs")
                        nc.vector.tensor_copy(out=rbs, in_=rb)
                        nc.gpsimd.tensor_tensor(
                            OcatT[0:64, p, s0 : s0 + 512],
                            OcatT[0:64, p, s0 : s0 + 512],
                            rbs[0:64, :],
                            mybir.AluOpType.mult,
                        )
                        nc.gpsimd.tensor_tensor(
                            OcatT[64:128, p, s0 : s0 + 512],
                            OcatT[64:128, p, s0 : s0 + 512],
                            rbs[64:128, :],
                            mybir.AluOpType.mult,
                        )

            # ---- P3: output projection (row-parallel partial) ----
            with (
                tc.tile_pool(name="p3", bufs=3) as p3,
                tc.tile_pool(name="p3w", bufs=1) as p3w,
                tc.tile_pool(name="ps3", bufs=4, space="PSUM") as ps3,
            ):
                wp_sb = p3w.tile([P, NPAIR, D], BF16, tag="wp")
                for p in range(NPAIR):
                    nc.sync.dma_start(wp_sb[:, p, :], wp_d[p])
                for st in range(NST):
                    for dc2 in range(2):
                        ps = ps3.tile([P, 512], F32, tag="y")
                        for p in range(NPAIR):
                            nc.tensor.matmul(
                                ps,
                                OcatT[:, p, st * P : (st + 1) * P],
                                wp_sb[:, p, dc2 * 512 : (dc2 + 1) * 512],
                                start=(p == 0),
                                stop=(p == NPAIR - 1),
                            )
                        yt = p3.tile([P, 512], F32, tag="yt")
                        nc.scalar.copy(out=yt, in_=ps)
                        nc.sync.dma_start(
                            y_d[st * P : (st + 1) * P, dc2 * 512 : (dc2 + 1) * 512],
                            yt,
                        )

    fix_extra_waits(nc)
    return nc


_NC = None


def _get_nc():
    global _NC
    if _NC is None:
        _NC = build_nc()
    return _NC


def _prep_core_inputs(x, Wq, Wk, Wv, Wp, core):
    b, hg = core // 2, core % 2
    hsl = slice(hg * 8, hg * 8 + 8)

    def prep_w(W):
        # [8, D, HD] -> [pair, dp, dc, (hip k)]
        a = W[hsl].reshape(NPAIR, 2, NDC, P, HD)
        return np.ascontiguousarray(
            a.transpose(0, 3, 2, 1, 4).reshape(NPAIR, P, NDC, P)
        ).astype(NPBF)

    r = np.arange(P)[:, None]
    c = np.arange(512)[None, :]
    masks = np.stack(
        [np.where(c - P * j >= r, 0.0, -1e30) for j in range(NS)]
    ).astype(np.float32)
    sel2 = np.zeros((65, P), dtype=NPBF)
    sel2[0, 0:64] = 1.0
    sel2[64, 64:128] = 1.0

    return {
        "x": np.ascontiguousarray(x[b]).astype(NPBF),
        "wq": prep_w(Wq),
        "wk": prep_w(Wk),
        "wv": prep_w(Wv),
        "wp": np.ascontiguousarray(
            Wp[hg * 512 : (hg + 1) * 512].reshape(NPAIR, P, D)
        ).astype(NPBF),
        "ident": np.eye(P, dtype=NPBF),
        "ones": np.ones((P, P), dtype=NPBF),
        "masks": masks,
        "sel2": sel2,
    }


def kernel(trace=False, **inputs):
    x = np.asarray(inputs["x"], dtype=np.float32)
    Wq = np.asarray(inputs["Wq"], dtype=np.float32)
    Wk = np.asarray(inputs["Wk"], dtype=np.float32)
    Wv = np.asarray(inputs["Wv"], dtype=np.float32)
    Wp = np.asarray(inputs["Wp"], dtype=np.float32)
    bp = np.asarray(inputs["bp"], dtype=np.float32)

    nc = _get_nc()
    in_maps = [_prep_core_inputs(x, Wq, Wk, Wv, Wp, c) for c in range(8)]
    res = run_bass_kernel_spmd(nc, in_maps, core_ids=list(range(8)), trace=trace)

    out = np.empty((B, S, D), dtype=np.float32)
    for b in range(B):
        out[b] = res.results[2 * b]["y"] + res.results[2 * b + 1]["y"] + bp
    if trace:
        return out, res
    return out


# revision 22
# speedup vs baseline: 1.4099x; 1.0285x over previous
"""Multi-head attention (B=4, S=2048, D=1024, H=16, causal) on 8 TRN2 NeuronCores.

Sharding: core c -> (batch b = c//2, head-group hg = c%2 of 8 heads).
Per core: QKV projections for its 8 heads (via on-chip transpose of x),
causal attention in transposed layout (scoresT[t, s]), softmax denominator
via an appended ones-column in the att@V matmul, then a row-parallel output
projection producing a partial [S, D]. Host sums the two head-group
partials per batch and adds the bias.

All matmul operands are bf16 (fp32 accumulation in PSUM). The att@V
stationary operand is padded to the full 128 columns (output rows 65-127
are never read) so every attention matmul exercises the whole PE array and
the HAM clock gate stays at full speed. The two heads of a pair are
processed together: score matmuls for head0/head1 are emitted adjacently
into distinct PE row-groups, one exp covers both heads' score tiles, and
causal masking is one DVE add of a precomputed [-1e30/triangle] mask per
diagonal tile. Softmax normalization computes 1/den as exp(-ln(den)) on
the scalar engine, expands it with one small broadcast matmul, and applies
it in place on the GpSimd engine after raw outputs are copied out.
"""

import sys

import numpy as np

for _p in ("/opt/trn_rl_repo", "/root/.axon_site/_ro/trn_rl_repo"):
    if _p not in sys.path:
        sys.path.append(_p)

import ml_dtypes

import concourse.bass as bass
import concourse.tile as tile
from concourse import mybir
from concourse.bass_utils import run_bass_kernel_spmd

F32 = mybir.dt.float32
BF16 = mybir.dt.bfloat16
NPBF = ml_dtypes.bfloat16

B, S, D, H, HD = 4, 2048, 1024, 16, 64
P = 128
NPAIR = 4  # head pairs per core (8 heads)
NS = S // 512  # 4 s-runs of 512
NST = S // P  # 16 s-tiles of 128
NDC = D // P  # 8 d-chunks

_WAIT_EXEMPT = {
    "InstEventSemaphore",
    "InstUnconditionalBranch",
    "InstCall",
    "InstRegisterMove",
}


def fix_extra_waits(nc):
    """TRN2 compute-instruction structs encode at most one semaphore wait.
    After Tile scheduling, move extra waits onto engine NOPs inserted just
    before the over-constrained instruction (same engine, final order)."""
    import copy

    # template InstNoOp per engine (nop() appends to the tail block; pop it)
    templates = {}

    def make_nop(engine):
        if engine not in templates:
            nc.engines[engine].nop()
            tail = nc.m.functions[0].blocks[-1]
            insts = tail.instructions
            templates[engine] = insts.pop()
            tail.instructions = insts
        nop = copy.deepcopy(templates[engine])
        nop.name = nc.get_next_instruction_name()
        return nop

    n_fixed = 0
    for fn in nc.m.functions:
        for blk in fn.blocks:
            out = []
            for inst in blk.instructions:
                si = getattr(inst, "sync_info", None)
                if (
                    type(inst).__name__ not in _WAIT_EXEMPT
                    and si is not None
                    and si.on_wait
                    and len(si.on_wait) > 1
                ):
                    waits = list(si.on_wait)
                    for w in waits[:-1]:
                        nop = make_nop(inst.engine)
                        nop.sync_info = mybir.SyncInfo(on_wait=[w], on_update=[])
                        out.append(nop)
                    si.on_wait = [waits[-1]]
                    n_fixed += 1
                out.append(inst)
            blk.instructions = out
    return n_fixed


def build_nc(reps=1):
    nc = bass.Bass()
    x_d = nc.dram_tensor("x", [S, D], BF16, kind="ExternalInput")
    wq_d = nc.dram_tensor("wq", [NPAIR, P, NDC, P], BF16, kind="ExternalInput")
    wk_d = nc.dram_tensor("wk", [NPAIR, P, NDC, P], BF16, kind="ExternalInput")
    wv_d = nc.dram_tensor("wv", [NPAIR, P, NDC, P], BF16, kind="ExternalInput")
    wp_d = nc.dram_tensor("wp", [NPAIR, P, D], BF16, kind="ExternalInput")
    ident_d = nc.dram_tensor("ident", [P, P], BF16, kind="ExternalInput")
    ones_d = nc.dram_tensor("ones", [P, P], BF16, kind="ExternalInput")
    masks_d = nc.dram_tensor("masks", [NS, P, 512], F32, kind="ExternalInput")
    sel2_d = nc.dram_tensor("sel2", [65, P], BF16, kind="ExternalInput")
    y_d = nc.dram_tensor("y", [S, D], F32, kind="ExternalOutput")

    import contextlib

    with tile.TileContext(nc) as tc:
        rep_ctx = tc.For_i(0, reps, 1) if reps > 1 else contextlib.nullcontext()
        with rep_ctx, tc.tile_pool(name="consts", bufs=1) as consts:
            ident = consts.tile([P, P], BF16, tag="ident")
            nc.sync.dma_start(ident, ident_d[:, :])
            sel2 = consts.tile([65, P], BF16, tag="sel2")
            nc.sync.dma_start(sel2, sel2_d[:, :])
            # denominator staging rows 0/64; rows 1-63 stay 1.0 (never
            # written) so ln->exp of them is finite where sel2 rows are zero
            dnP = consts.tile([65, S], F32, tag="dnP")
            nc.gpsimd.memset(dnP, 1.0)
            masks = consts.tile([P, NS, 512], F32, tag="masks")
            for j in range(NS):
                nc.sync.dma_start(masks[:, j, :], masks_d[j])
            # x transposed: [d-part, d-chunk, t]
            xT = consts.tile([P, NDC, S], BF16, tag="xT")
            # attention output (raw, then normalized in place), transposed:
            # [pair-hk part, pair, s]
            OcatT = consts.tile([P, NPAIR, S], BF16, tag="OcatT")

            # ---- P0: build xT via PE transposes ----
            with (
                tc.tile_pool(name="p0", bufs=3) as p0,
                tc.tile_pool(name="ps0", bufs=2, space="PSUM") as ps0,
            ):
                for st in range(NST):
                    xin = p0.tile([P, D], BF16, tag="xin")
                    nc.sync.dma_start(xin, x_d[st * P : (st + 1) * P, :])
                    for dc in range(NDC):
                        pt = ps0.tile([P, P], BF16, tag="tr")
                        nc.tensor.transpose(
                            pt, xin[:, dc * P : (dc + 1) * P], ident
                        )
                        nc.vector.tensor_copy(
                            out=xT[:, dc, st * P : (st + 1) * P], in_=pt
                        )

            # ---- P1+P2: per head-pair projections + attention ----
            with (
                tc.tile_pool(name="pw", bufs=2) as pw,
                tc.tile_pool(name="pqk", bufs=2) as pqk,
                tc.tile_pool(name="pvt", bufs=2) as pvt,
                tc.tile_pool(name="pvp", bufs=2) as pvp,
                tc.tile_pool(name="pex", bufs=2) as pex,
                tc.tile_pool(name="pnrm", bufs=1) as pnrm,
                tc.tile_pool(name="psM", bufs=2, space="PSUM") as psM,
                tc.tile_pool(name="psS", bufs=2, space="PSUM") as psS,
                tc.tile_pool(name="psO", bufs=1, space="PSUM") as psO,
            ):
                def qkv_units(p):
                    """Projection work for pair p as a list of closures, so
                    it can be interleaved into the previous pair's attention
                    loop (keeps the PE densely fed with full-array matmuls
                    while the scalar engine works through the exps)."""
                    wq_sb = pw.tile([P, NDC, P], BF16, tag="wq")
                    wk_sb = pw.tile([P, NDC, P], BF16, tag="wk")
                    wv_sb = pw.tile([P, NDC, P], BF16, tag="wv")
                    QT = pqk.tile([P, S], BF16, tag="QT")
                    KT = pqk.tile([P, S], BF16, tag="KT")
                    # V natural layout, padded to the full 128 weight columns
                    # per head: [t-part, tt, head, (64 V | 1 ones | 63 junk)].
                    # Matmul output rows 65-127 are never read, so the pad
                    # columns need no initialization.
                    Vp = pvp.tile([P, NST, 2, P], BF16, tag="Vp")
                    units = []

                    def u_dma():
                        nc.sync.dma_start(wq_sb, wq_d[p])
                        nc.sync.dma_start(wk_sb, wk_d[p])
                        nc.sync.dma_start(wv_sb, wv_d[p])
                        nc.sync.dma_start(
                            Vp[:, :, :, 64:65],
                            ones_d[:, 0:32].rearrange(
                                "p (t two one) -> p t two one", two=2, one=1
                            ),
                        )

                    units.append(u_dma)

                    def u_proj(w_sb, dst, s0):
                        ps = psM.tile([P, 512], F32, tag="mm512")
                        for dc in range(NDC):
                            nc.tensor.matmul(
                                ps,
                                w_sb[:, dc],
                                xT[:, dc, s0 : s0 + 512],
                                start=(dc == 0),
                                stop=(dc == NDC - 1),
                            )
                        nc.vector.tensor_copy(out=dst[:, s0 : s0 + 512], in_=ps)

                    vts = {}

                    def u_vproj(sc):
                        s0 = sc * 512
                        VT = pvt.tile([P, 512], BF16, tag="VT")
                        ps = psM.tile([P, 512], F32, tag="mm512")
                        for dc in range(NDC):
                            nc.tensor.matmul(
                                ps,
                                wv_sb[:, dc],
                                xT[:, dc, s0 : s0 + 512],
                                start=(dc == 0),
                                stop=(dc == NDC - 1),
                            )
                        nc.vector.tensor_copy(out=VT, in_=ps)
                        vts[sc] = VT

                    def u_vtrans(sc):
                        VT = vts.pop(sc)
                        for tq in range(4):
                            tt = sc * 4 + tq
                            ptv = psM.tile([P, 1024], BF16, tag="mm512")
                            nc.tensor.transpose(
                                ptv[:, 0:P], VT[:, tq * P : (tq + 1) * P], ident
                            )
                            for h in (0, 1):
                                nc.vector.tensor_copy(
                                    out=Vp[:, tt, h, 0:64],
                                    in_=ptv[:, 64 * h : 64 * h + 64],
                                )

                    for sc in range(NS):
                        units.append(lambda sc=sc: u_proj(wq_sb, QT, sc * 512))
                        units.append(lambda sc=sc: u_proj(wk_sb, KT, sc * 512))
                        units.append(lambda sc=sc: u_vproj(sc))
                        units.append(lambda sc=sc: u_vtrans(sc))
                    return units, QT, KT, Vp

                units0, QT, KT, Vp = qkv_units(0)
                for u in units0:
                    u()
                for p in range(NPAIR):
                    pending = []
                    if p + 1 < NPAIR:
                        pending, QTn, KTn, Vpn = qkv_units(p + 1)
                    n_iters = sum(4 * (sr + 1) for sr in range(NS))
                    done = 0
                    it = 0

                    # attention: both heads per t-tile, row-tiled scores
                    for sr in range(NS):
                        s0 = sr * 512
                        n_t = 4 * (sr + 1)
                        po0 = psO.tile([P, 512], F32, tag="po0", name="po0")
                        po1 = psO.tile([P, 512], F32, tag="po1", name="po1")
                        ets = {}

                        def emit_attv(tt):
                            et = ets.pop(tt)
                            nc.tensor.matmul(
                                po0,
                                Vp[:, tt, 0],
                                et[:, 0],
                                start=(tt == 0),
                                stop=(tt == n_t - 1),
                            )
                            nc.tensor.matmul(
                                po1,
                                Vp[:, tt, 1],
                                et[:, 1],
                                start=(tt == 0),
                                stop=(tt == n_t - 1),
                            )

                        for tt in range(n_t):
                            pss = psS.tile([P, 2, 512], F32, tag="s", name="s")
                            for i in (0, 1):
                                nc.tensor.matmul(
                                    pss[:, i],
                                    KT[64 * i : 64 * i + 64, tt * P : (tt + 1) * P],
                                    QT[64 * i : 64 * i + 64, s0 : s0 + 512],
                                    start=True,
                                    stop=True,
                                )
                            j = tt - 4 * sr
                            if j >= 0:
                                w = P * (j + 1)
                                for i in (0, 1):
                                    nc.vector.tensor_tensor(
                                        pss[:, i, 0:w],
                                        pss[:, i, 0:w],
                                        masks[:, j, 0:w],
                                        mybir.AluOpType.add,
                                    )
                            et = pex.tile([P, 2, 512], BF16, tag="e", name="e")
                            nc.scalar.activation(
                                out=et,
                                in_=pss,
                                func=mybir.ActivationFunctionType.Exp,
                                scale=float(HD**-0.5),
                            )
                            ets[tt] = et
                            if tt >= 1:
                                emit_attv(tt - 1)
                            it += 1
                            if pending:
                                want = (it * len(pending)) // n_iters
                                while done < want:
                                    pending[done]()
                                    done += 1
                        emit_attv(n_t - 1)

                        # raw attention outputs; normalization is applied
                        # in place before the output projection reads them
                        nc.vector.tensor_copy(
                            out=OcatT[0:64, p, s0 : s0 + 512], in_=po0[0:64, :]
                        )
                        nc.vector.tensor_copy(
                            out=dnP[0:1, s0 : s0 + 512], in_=po0[64:65, :]
                        )
                        nc.vector.tensor_copy(
                            out=OcatT[64:128, p, s0 : s0 + 512], in_=po1[0:64, :]
                        )
                        nc.vector.tensor_copy(
                            out=dnP[64:65, s0 : s0 + 512], in_=po1[64:65, :]
                        )
                        # 1/den = exp(-ln(den)) on the scalar engine (the DVE
                        # reciprocal is ~6.5 ns/elem; ACT is ~1.1)
                        lnt = pnrm.tile([65, 512], F32, tag="lnt")
                        nc.scalar.activation(
                            out=lnt,
                            in_=dnP[0:65, s0 : s0 + 512],
                            func=mybir.ActivationFunctionType.Ln,
                        )
                        rcr = pnrm.tile([65, 512], BF16, tag="rcr")
                        nc.scalar.activation(
                            out=rcr,
                            in_=lnt,
                            func=mybir.ActivationFunctionType.Exp,
                            scale=-1.0,
                        )
                        rb = psM.tile([P, 512], F32, tag="mm512")
                        nc.tensor.matmul(rb, sel2, rcr, start=True, stop=True)
                        rbs = pnrm.tile([P, 512], BF16, tag="rbs")
                        nc.vector.tensor_copy(out=rbs, in_=rb)
                        nc.gpsimd.tensor_tensor(
                            OcatT[0:64, p, s0 : s0 + 512],
                            OcatT[0:64, p, s0 : s0 + 512],
                            rbs[0:64, :],
                            mybir.AluOpType.mult,
                        )
                        nc.gpsimd.tensor_tensor(
                            OcatT[64:128, p, s0 : s0 + 512],
                            OcatT[64:128, p, s0 : s0 + 512],
                            rbs[64:128, :],
                            mybir.AluOpType.mult,
                        )
                    while done < len(pending):
                        pending[done]()
                        done += 1
                    if p + 1 < NPAIR:
                        QT, KT, Vp = QTn, KTn, Vpn

            # ---- P3: output projection (row-parallel partial) ----
            with (
                tc.tile_pool(name="p3", bufs=3) as p3,
                tc.tile_pool(name="p3w", bufs=1) as p3w,
                tc.tile_pool(name="ps3", bufs=4, space="PSUM") as ps3,
            ):
                wp_sb = p3w.tile([P, NPAIR, D], BF16, tag="wp")
                for p in range(NPAIR):
                    nc.sync.dma_start(wp_sb[:, p, :], wp_d[p])
                for st in range(NST):
                    for dc2 in range(2):
                        ps = ps3.tile([P, 512], F32, tag="y")
                        for p in range(NPAIR):
                            nc.tensor.matmul(
                                ps,
                                OcatT[:, p, st * P : (st + 1) * P],
                                wp_sb[:, p, dc2 * 512 : (dc2 + 1) * 512],
                                start=(p == 0),
                                stop=(p == NPAIR - 1),
                            )
                        yt = p3.tile([P, 512], F32, tag="yt")
                        nc.scalar.copy(out=yt, in_=ps)
                        nc.sync.dma_start(
                            y_d[st * P : (st + 1) * P, dc2 * 512 : (dc2 + 1) * 512],
                            yt,
                        )

    fix_extra_waits(nc)
    return nc


_NC = None


def _get_nc():
    global _NC
    if _NC is None:
        _NC = build_nc()
    return _NC


def _prep_core_inputs(x, Wq, Wk, Wv, Wp, core):
    b, hg = core // 2, core % 2
    hsl = slice(hg * 8, hg * 8 + 8)

    def prep_w(W):
        # [8, D, HD] -> [pair, dp, dc, (hip k)]
        a = W[hsl].reshape(NPAIR, 2, NDC, P, HD)
        return np.ascontiguousarray(
            a.transpose(0, 3, 2, 1, 4).reshape(NPAIR, P, NDC, P)
        ).astype(NPBF)

    r = np.arange(P)[:, None]
    c = np.arange(512)[None, :]
    masks = np.stack(
        [np.where(c - P * j >= r, 0.0, -1e30) for j in range(NS)]
    ).astype(np.float32)
    sel2 = np.zeros((65, P), dtype=NPBF)
    sel2[0, 0:64] = 1.0
    sel2[64, 64:128] = 1.0

    return {
        "x": np.ascontiguousarray(x[b]).astype(NPBF),
        "wq": prep_w(Wq),
        "wk": prep_w(Wk),
        "wv": prep_w(Wv),
        "wp": np.ascontiguousarray(
            Wp[hg * 512 : (hg + 1) * 512].reshape(NPAIR, P, D)
        ).astype(NPBF),
        "ident": np.eye(P, dtype=NPBF),
        "ones": np.ones((P, P), dtype=NPBF),
        "masks": masks,
        "sel2": sel2,
    }


def kernel(trace=False, **inputs):
    x = np.asarray(inputs["x"], dtype=np.float32)
    Wq = np.asarray(inputs["Wq"], dtype=np.float32)
    Wk = np.asarray(inputs["Wk"], dtype=np.float32)
    Wv = np.asarray(inputs["Wv"], dtype=np.float32)
    Wp = np.asarray(inputs["Wp"], dtype=np.float32)
    bp = np.asarray(inputs["bp"], dtype=np.float32)

    nc = _get_nc()
    in_maps = [_prep_core_inputs(x, Wq, Wk, Wv, Wp, c) for c in range(8)]
    res = run_bass_kernel_spmd(nc, in_maps, core_ids=list(range(8)), trace=trace)

    out = np.empty((B, S, D), dtype=np.float32)
    for b in range(B):
        out[b] = res.results[2 * b]["y"] + res.results[2 * b + 1]["y"] + bp
    if trace:
        return out, res
    return out


# revision 26
# speedup vs baseline: 1.4357x; 1.0183x over previous
"""Multi-head attention (B=4, S=2048, D=1024, H=16, causal) on 8 TRN2 NeuronCores.

Sharding: core c -> (batch b = c//2, head-group hg = c%2 of 8 heads).
Per core: QKV projections for its 8 heads (via on-chip transpose of x),
causal attention in transposed layout (scoresT[t, s]), softmax denominator
via an appended ones-column in the att@V matmul, then a row-parallel output
projection producing a partial [S, D]. Host sums the two head-group
partials per batch and adds the bias.

All matmul operands are bf16 (fp32 accumulation in PSUM). The att@V
stationary operand is padded to the full 128 columns (output rows 65-127
are never read) so every attention matmul exercises the whole PE array and
the HAM clock gate stays at full speed. The two heads of a pair are
processed together: score matmuls for head0/head1 are emitted adjacently
into distinct PE row-groups, one exp covers both heads' score tiles, and
causal masking is one DVE add of a precomputed [-1e30/triangle] mask per
diagonal tile. Softmax normalization computes 1/den as exp(-ln(den)) on
the scalar engine, expands it with one small broadcast matmul, and applies
it in place on the GpSimd engine after raw outputs are copied out.
"""

import sys

import numpy as np

for _p in ("/opt/trn_rl_repo", "/root/.axon_site/_ro/trn_rl_repo"):
    if _p not in sys.path:
        sys.path.append(_p)

import ml_dtypes

import concourse.bass as bass
import concourse.tile as tile
from concourse import mybir
from concourse.bass_utils import run_bass_kernel_spmd

F32 = mybir.dt.float32
BF16 = mybir.dt.bfloat16
NPBF = ml_dtypes.bfloat16

B, S, D, H, HD = 4, 2048, 1024, 16, 64
P = 128
NPAIR = 4  # head pairs per core (8 heads)
NS = S // 512  # 4 s-runs of 512
NST = S // P  # 16 s-tiles of 128
NDC = D // P  # 8 d-chunks

_WAIT_EXEMPT = {
    "InstEventSemaphore",
    "InstUnconditionalBranch",
    "InstCall",
    "InstRegisterMove",
}


def fix_extra_waits(nc):
    """TRN2 compute-instruction structs encode at most one semaphore wait.
    After Tile scheduling, move extra waits onto engine NOPs inserted just
    before the over-constrained instruction (same engine, final order)."""
    import copy

    # template InstNoOp per engine (nop() appends to the tail block; pop it)
    templates = {}

    def make_nop(engine):
        if engine not in templates:
            nc.engines[engine].nop()
            tail = nc.m.functions[0].blocks[-1]
            insts = tail.instructions
            templates[engine] = insts.pop()
            tail.instructions = insts
        nop = copy.deepcopy(templates[engine])
        nop.name = nc.get_next_instruction_name()
        return nop

    n_fixed = 0
    for fn in nc.m.functions:
        for blk in fn.blocks:
            out = []
            for inst in blk.instructions:
                si = getattr(inst, "sync_info", None)
                if (
                    type(inst).__name__ not in _WAIT_EXEMPT
                    and si is not None
                    and si.on_wait
                    and len(si.on_wait) > 1
                ):
                    waits = list(si.on_wait)
                    for w in waits[:-1]:
                        nop = make_nop(inst.engine)
                        nop.sync_info = mybir.SyncInfo(on_wait=[w], on_update=[])
                        out.append(nop)
                    si.on_wait = [waits[-1]]
                    n_fixed += 1
                out.append(inst)
            blk.instructions = out
    return n_fixed


def build_nc(reps=1):
    nc = bass.Bass()
    x_d = nc.dram_tensor("x", [S, D], BF16, kind="ExternalInput")
    wq_d = nc.dram_tensor("wq", [NPAIR, P, NDC, P], BF16, kind="ExternalInput")
    wk_d = nc.dram_tensor("wk", [NPAIR, P, NDC, P], BF16, kind="ExternalInput")
    wv_d = nc.dram_tensor("wv", [NPAIR, P, NDC, P], BF16, kind="ExternalInput")
    wp_d = nc.dram_tensor("wp", [NPAIR, P, D], BF16, kind="ExternalInput")
    ident_d = nc.dram_tensor("ident", [P, P], BF16, kind="ExternalInput")
    ones_d = nc.dram_tensor("ones", [P, P], BF16, kind="ExternalInput")
    masks_d = nc.dram_tensor("masks", [NS, P, 512], F32, kind="ExternalInput")
    sel2_d = nc.dram_tensor("sel2", [65, P], BF16, kind="ExternalInput")
    y_d = nc.dram_tensor("y", [S, D], F32, kind="ExternalOutput")

    import contextlib

    with tile.TileContext(nc) as tc:
        rep_ctx = tc.For_i(0, reps, 1) if reps > 1 else contextlib.nullcontext()
        with rep_ctx, tc.tile_pool(name="consts", bufs=1) as consts:
            ident = consts.tile([P, P], BF16, tag="ident")
            nc.sync.dma_start(ident, ident_d[:, :])
            sel2 = consts.tile([65, P], BF16, tag="sel2")
            nc.sync.dma_start(sel2, sel2_d[:, :])
            # denominator staging rows 0/64; rows 1-63 stay 1.0 (never
            # written) so ln->exp of them is finite where sel2 rows are zero
            dnP = consts.tile([65, S], F32, tag="dnP")
            nc.gpsimd.memset(dnP, 1.0)
            masks = consts.tile([P, NS, 512], F32, tag="masks")
            for j in range(NS):
                nc.sync.dma_start(masks[:, j, :], masks_d[j])
            # x transposed: [d-part, d-chunk, t]
            xT = consts.tile([P, NDC, S], BF16, tag="xT")
            # attention output (raw, then normalized in place), transposed:
            # [pair-hk part, pair, s]
            OcatT = consts.tile([P, NPAIR, S], BF16, tag="OcatT")

            # ---- P0: build xT via PE transposes ----
            with (
                tc.tile_pool(name="p0", bufs=3) as p0,
                tc.tile_pool(name="ps0", bufs=2, space="PSUM") as ps0,
            ):
                for st in range(NST):
                    xin = p0.tile([P, D], BF16, tag="xin")
                    nc.sync.dma_start(xin, x_d[st * P : (st + 1) * P, :])
                    for dc in range(NDC):
                        pt = ps0.tile([P, P], BF16, tag="tr")
                        nc.tensor.transpose(
                            pt, xin[:, dc * P : (dc + 1) * P], ident
                        )
                        nc.vector.tensor_copy(
                            out=xT[:, dc, st * P : (st + 1) * P], in_=pt
                        )

            # ---- P1+P2: per head-pair projections + attention ----
            with (
                tc.tile_pool(name="pw", bufs=2) as pw,
                tc.tile_pool(name="pqk", bufs=2) as pqk,
                tc.tile_pool(name="pvt", bufs=2) as pvt,
                tc.tile_pool(name="pvp", bufs=2) as pvp,
                tc.tile_pool(name="pex", bufs=2) as pex,
                tc.tile_pool(name="pnrm", bufs=4) as pnrm,
                tc.tile_pool(name="psM", bufs=2, space="PSUM") as psM,
                tc.tile_pool(name="psS", bufs=2, space="PSUM") as psS,
                tc.tile_pool(name="psO", bufs=1, space="PSUM") as psO,
            ):
                def qkv_units(p):
                    """Projection work for pair p as a list of closures, so
                    it can be interleaved into the previous pair's attention
                    loop (keeps the PE densely fed with full-array matmuls
                    while the scalar engine works through the exps)."""
                    wq_sb = pw.tile([P, NDC, P], BF16, tag="wq")
                    wk_sb = pw.tile([P, NDC, P], BF16, tag="wk")
                    wv_sb = pw.tile([P, NDC, P], BF16, tag="wv")
                    QT = pqk.tile([P, S], BF16, tag="QT")
                    KT = pqk.tile([P, S], BF16, tag="KT")
                    # V natural layout, padded to the full 128 weight columns
                    # per head: [t-part, tt, head, (64 V | 1 ones | 63 junk)].
                    # Matmul output rows 65-127 are never read, so the pad
                    # columns need no initialization.
                    Vp = pvp.tile([P, NST, 2, P], BF16, tag="Vp")
                    units = []

                    def u_dma():
                        nc.sync.dma_start(wq_sb, wq_d[p])
                        nc.sync.dma_start(wk_sb, wk_d[p])
                        nc.sync.dma_start(wv_sb, wv_d[p])
                        nc.sync.dma_start(
                            Vp[:, :, :, 64:65],
                            ones_d[:, 0:32].rearrange(
                                "p (t two one) -> p t two one", two=2, one=1
                            ),
                        )

                    units.append(u_dma)

                    def u_proj(w_sb, dst, s0):
                        ps = psM.tile([P, 512], F32, tag="mm512")
                        for dc in range(NDC):
                            nc.tensor.matmul(
                                ps,
                                w_sb[:, dc],
                                xT[:, dc, s0 : s0 + 512],
                                start=(dc == 0),
                                stop=(dc == NDC - 1),
                            )
                        nc.vector.tensor_copy(out=dst[:, s0 : s0 + 512], in_=ps)

                    vts = {}

                    def u_vproj(sc):
                        s0 = sc * 512
                        VT = pvt.tile([P, 512], BF16, tag="VT")
                        ps = psM.tile([P, 512], F32, tag="mm512")
                        for dc in range(NDC):
                            nc.tensor.matmul(
                                ps,
                                wv_sb[:, dc],
                                xT[:, dc, s0 : s0 + 512],
                                start=(dc == 0),
                                stop=(dc == NDC - 1),
                            )
                        nc.vector.tensor_copy(out=VT, in_=ps)
                        vts[sc] = VT

                    def u_vtrans(sc):
                        VT = vts.pop(sc)
                        for tq in range(4):
                            tt = sc * 4 + tq
                            ptv = psM.tile([P, 1024], BF16, tag="mm512")
                            nc.tensor.transpose(
                                ptv[:, 0:P], VT[:, tq * P : (tq + 1) * P], ident
                            )
                            for h in (0, 1):
                                nc.vector.tensor_copy(
                                    out=Vp[:, tt, h, 0:64],
                                    in_=ptv[:, 64 * h : 64 * h + 64],
                                )

                    for sc in range(NS):
                        units.append(lambda sc=sc: u_proj(wq_sb, QT, sc * 512))
                        units.append(lambda sc=sc: u_proj(wk_sb, KT, sc * 512))
                        units.append(lambda sc=sc: u_vproj(sc))
                        if sc >= 1:
                            units.append(lambda sc=sc: u_vtrans(sc - 1))
                    units.append(lambda: u_vtrans(NS - 1))
                    return units, QT, KT, Vp

                from collections import deque

                pending = deque()
                units0, QT, KT, Vp = qkv_units(0)
                for u in units0:
                    u()
                for p in range(NPAIR):
                    if p + 1 < NPAIR:
                        un, QTn, KTn, Vpn = qkv_units(p + 1)
                        pending.extend(un)

                    # attention: both heads per t-tile, row-tiled scores
                    for sr in range(NS):
                        s0 = sr * 512
                        n_t = 4 * (sr + 1)
                        po0 = psO.tile([P, 512], F32, tag="po0", name="po0")
                        po1 = psO.tile([P, 512], F32, tag="po1", name="po1")
                        ets = {}

                        def emit_attv(tt):
                            et = ets.pop(tt)
                            nc.tensor.matmul(
                                po0,
                                Vp[:, tt, 0],
                                et[:, 0],
                                start=(tt == 0),
                                stop=(tt == n_t - 1),
                            )
                            nc.tensor.matmul(
                                po1,
                                Vp[:, tt, 1],
                                et[:, 1],
                                start=(tt == 0),
                                stop=(tt == n_t - 1),
                            )

                        for tt in range(n_t):
                            pss = psS.tile([P, 2, 512], F32, tag="s", name="s")
                            for i in (0, 1):
                                nc.tensor.matmul(
                                    pss[:, i],
                                    KT[64 * i : 64 * i + 64, tt * P : (tt + 1) * P],
                                    QT[64 * i : 64 * i + 64, s0 : s0 + 512],
                                    start=True,
                                    stop=True,
                                )
                            j = tt - 4 * sr
                            if j >= 0:
                                w = P * (j + 1)
                                for i in (0, 1):
                                    nc.vector.tensor_tensor(
                                        pss[:, i, 0:w],
                                        pss[:, i, 0:w],
                                        masks[:, j, 0:w],
                                        mybir.AluOpType.add,
                                    )
                            et = pex.tile([P, 2, 512], BF16, tag="e", name="e")
                            nc.scalar.activation(
                                out=et,
                                in_=pss,
                                func=mybir.ActivationFunctionType.Exp,
                                scale=float(HD**-0.5),
                            )
                            ets[tt] = et
                            if tt >= 1:
                                emit_attv(tt - 1)
                            if pending:
                                pending.popleft()()
                        emit_attv(n_t - 1)

                        # raw attention outputs; normalization is applied
                        # in place before the output projection reads them
                        nc.vector.tensor_copy(
                            out=OcatT[0:64, p, s0 : s0 + 512], in_=po0[0:64, :]
                        )
                        nc.vector.tensor_copy(
                            out=dnP[0:1, s0 : s0 + 512], in_=po0[64:65, :]
                        )
                        nc.vector.tensor_copy(
                            out=OcatT[64:128, p, s0 : s0 + 512], in_=po1[0:64, :]
                        )
                        nc.vector.tensor_copy(
                            out=dnP[64:65, s0 : s0 + 512], in_=po1[64:65, :]
                        )
                        # 1/den = exp(-ln(den)) on the scalar engine (the DVE
                        # reciprocal is ~6.5 ns/elem; ACT is ~1.1)
                        lnt = pnrm.tile([65, 512], F32, tag="lnt")
                        nc.scalar.activation(
                            out=lnt,
                            in_=dnP[0:65, s0 : s0 + 512],
                            func=mybir.ActivationFunctionType.Ln,
                        )
                        rcr = pnrm.tile([65, 512], BF16, tag="rcr")
                        nc.scalar.activation(
                            out=rcr,
                            in_=lnt,
                            func=mybir.ActivationFunctionType.Exp,
                            scale=-1.0,
                        )

                        def u_norm(p=p, s0=s0, rcr=rcr):
                            # deferred so the broadcast matmul's ACT-chain
                            # dependency never stalls the PE at s-run tails
                            rb = psM.tile([P, 512], F32, tag="mm512")
                            nc.tensor.matmul(rb, sel2, rcr, start=True, stop=True)
                            rbs = pnrm.tile([P, 512], BF16, tag="rbs")
                            nc.vector.tensor_copy(out=rbs, in_=rb)
                            nc.gpsimd.tensor_tensor(
                                OcatT[0:64, p, s0 : s0 + 512],
                                OcatT[0:64, p, s0 : s0 + 512],
                                rbs[0:64, :],
                                mybir.AluOpType.mult,
                            )
                            nc.gpsimd.tensor_tensor(
                                OcatT[64:128, p, s0 : s0 + 512],
                                OcatT[64:128, p, s0 : s0 + 512],
                                rbs[64:128, :],
                                mybir.AluOpType.mult,
                            )

                        pending.append(u_norm)
                    if p + 1 < NPAIR:
                        QT, KT, Vp = QTn, KTn, Vpn
                while pending:
                    pending.popleft()()

            # ---- P3: output projection (row-parallel partial) ----
            with (
                tc.tile_pool(name="p3", bufs=3) as p3,
                tc.tile_pool(name="p3w", bufs=1) as p3w,
                tc.tile_pool(name="ps3", bufs=4, space="PSUM") as ps3,
            ):
                wp_sb = p3w.tile([P, NPAIR, D], BF16, tag="wp")
                for p in range(NPAIR):
                    nc.sync.dma_start(wp_sb[:, p, :], wp_d[p])
                for st in range(NST):
                    for dc2 in range(2):
                        ps = ps3.tile([P, 512], F32, tag="y")
                        for p in range(NPAIR):
                            nc.tensor.matmul(
                                ps,
                                OcatT[:, p, st * P : (st + 1) * P],
                                wp_sb[:, p, dc2 * 512 : (dc2 + 1) * 512],
                                start=(p == 0),
                                stop=(p == NPAIR - 1),
                            )
                        yt = p3.tile([P, 512], F32, tag="yt")
                        nc.scalar.copy(out=yt, in_=ps)
                        nc.sync.dma_start(
                            y_d[st * P : (st + 1) * P, dc2 * 512 : (dc2 + 1) * 512],
                            yt,
                        )

    fix_extra_waits(nc)
    return nc


_NC = None


def _get_nc():
    global _NC
    if _NC is None:
        _NC = build_nc()
    return _NC


def _prep_core_inputs(x, Wq, Wk, Wv, Wp, core):
    b, hg = core // 2, core % 2
    hsl = slice(hg * 8, hg * 8 + 8)

    def prep_w(W):
        # [8, D, HD] -> [pair, dp, dc, (hip k)]
        a = W[hsl].reshape(NPAIR, 2, NDC, P, HD)
        return np.ascontiguousarray(
            a.transpose(0, 3, 2, 1, 4).reshape(NPAIR, P, NDC, P)
        ).astype(NPBF)

    r = np.arange(P)[:, None]
    c = np.arange(512)[None, :]
    masks = np.stack(
        [np.where(c - P * j >= r, 0.0, -1e30) for j in range(NS)]
    ).astype(np.float32)
    sel2 = np.zeros((65, P), dtype=NPBF)
    sel2[0, 0:64] = 1.0
    sel2[64, 64:128] = 1.0

    return {
        "x": np.ascontiguousarray(x[b]).astype(NPBF),
        "wq": prep_w(Wq),
        "wk": prep_w(Wk),
        "wv": prep_w(Wv),
        "wp": np.ascontiguousarray(
            Wp[hg * 512 : (hg + 1) * 512].reshape(NPAIR, P, D)
        ).astype(NPBF),
        "ident": np.eye(P, dtype=NPBF),
        "ones": np.ones((P, P), dtype=NPBF),
        "masks": masks,
        "sel2": sel2,
    }


def kernel(trace=False, **inputs):
    x = np.asarray(inputs["x"], dtype=np.float32)
    Wq = np.asarray(inputs["Wq"], dtype=np.float32)
    Wk = np.asarray(inputs["Wk"], dtype=np.float32)
    Wv = np.asarray(inputs["Wv"], dtype=np.float32)
    Wp = np.asarray(inputs["Wp"], dtype=np.float32)
    bp = np.asarray(inputs["bp"], dtype=np.float32)

    nc = _get_nc()
    in_maps = [_prep_core_inputs(x, Wq, Wk, Wv, Wp, c) for c in range(8)]
    res = run_bass_kernel_spmd(nc, in_maps, core_ids=list(range(8)), trace=trace)

    out = np.empty((B, S, D), dtype=np.float32)
    for b in range(B):
        out[b] = res.results[2 * b]["y"] + res.results[2 * b + 1]["y"] + bp
    if trace:
        return out, res
    return out


# revision 34
# speedup vs baseline: 1.6042x; 1.1174x over previous
"""Multi-head attention (B=4, S=2048, D=1024, H=16, causal) on 8 TRN2 NeuronCores.

Sharding: core c -> (batch b = c//2, head-group hg = c%2 of 8 heads).
Per core: QKV projections for its 8 heads (via on-chip transpose of x),
causal attention in transposed layout (scoresT[t, s]), softmax denominator
via an appended ones-column in the att@V matmul, then a row-parallel output
projection producing a partial [S, D]. Host sums the two head-group
partials per batch and adds the bias.

All matmul operands are bf16 (fp32 accumulation in PSUM). The att@V
stationary operand is padded to the full 128 columns (output rows 65-127
are never read) so every attention matmul exercises the whole PE array and
the HAM clock gate stays at full speed. The two heads of a pair are
processed together: score matmuls for head0/head1 are emitted adjacently
into distinct PE row-groups, one exp covers both heads' score tiles, and
causal masking is one DVE add of a precomputed [-1e30/triangle] mask per
diagonal tile. Softmax normalization computes 1/den as exp(-ln(den)) on
the scalar engine, expands it with one small broadcast matmul, and applies
it in place on the GpSimd engine after raw outputs are copied out.
"""

import sys

import numpy as np

for _p in ("/opt/trn_rl_repo", "/root/.axon_site/_ro/trn_rl_repo"):
    if _p not in sys.path:
        sys.path.append(_p)

import ml_dtypes

import concourse.bass as bass
import concourse.tile as tile
from concourse import mybir
from concourse.bass_utils import run_bass_kernel_spmd

F32 = mybir.dt.float32
BF16 = mybir.dt.bfloat16
NPBF = ml_dtypes.bfloat16

B, S, D, H, HD = 4, 2048, 1024, 16, 64
P = 128
NPAIR = 4  # head pairs per core (8 heads)
NS = S // 512  # 4 s-runs of 512
NST = S // P  # 16 s-tiles of 128
NDC = D // P  # 8 d-chunks

_WAIT_EXEMPT = {
    "InstEventSemaphore",
    "InstUnconditionalBranch",
    "InstCall",
    "InstRegisterMove",
}


def fix_extra_waits(nc):
    """TRN2 compute-instruction structs encode at most one semaphore wait.
    After Tile scheduling, move extra waits onto engine NOPs inserted just
    before the over-constrained instruction (same engine, final order)."""
    import copy

    # template InstNoOp per engine (nop() appends to the tail block; pop it)
    templates = {}

    def make_nop(engine):
        if engine not in templates:
            nc.engines[engine].nop()
            tail = nc.m.functions[0].blocks[-1]
            insts = tail.instructions
            templates[engine] = insts.pop()
            tail.instructions = insts
        nop = copy.deepcopy(templates[engine])
        nop.name = nc.get_next_instruction_name()
        return nop

    n_fixed = 0
    for fn in nc.m.functions:
        for blk in fn.blocks:
            out = []
            for inst in blk.instructions:
                si = getattr(inst, "sync_info", None)
                if (
                    type(inst).__name__ not in _WAIT_EXEMPT
                    and si is not None
                    and si.on_wait
                    and len(si.on_wait) > 1
                ):
                    waits = list(si.on_wait)
                    for w in waits[:-1]:
                        nop = make_nop(inst.engine)
                        nop.sync_info = mybir.SyncInfo(on_wait=[w], on_update=[])
                        out.append(nop)
                    si.on_wait = [waits[-1]]
                    n_fixed += 1
                out.append(inst)
            blk.instructions = out
    return n_fixed


def build_nc(reps=1):
    nc = bass.Bass()
    x_d = nc.dram_tensor("x", [S, D], BF16, kind="ExternalInput")
    wq_d = nc.dram_tensor("wq", [NPAIR, P, NDC, P], BF16, kind="ExternalInput")
    wk_d = nc.dram_tensor("wk", [NPAIR, P, NDC, P], BF16, kind="ExternalInput")
    wv_d = nc.dram_tensor("wv", [NPAIR, P, NDC, P], BF16, kind="ExternalInput")
    wp_d = nc.dram_tensor("wp", [NPAIR, P, D], BF16, kind="ExternalInput")
    ident_d = nc.dram_tensor("ident", [P, P], BF16, kind="ExternalInput")
    ones_d = nc.dram_tensor("ones", [P, P], BF16, kind="ExternalInput")
    trimask_d = nc.dram_tensor("trimask", [P, P], F32, kind="ExternalInput")
    sel2_d = nc.dram_tensor("sel2", [65, P], BF16, kind="ExternalInput")
    y_d = nc.dram_tensor("y", [S, D], F32, kind="ExternalOutput")

    import contextlib

    with tile.TileContext(nc) as tc:
        rep_ctx = tc.For_i(0, reps, 1) if reps > 1 else contextlib.nullcontext()
        with rep_ctx, tc.tile_pool(name="consts", bufs=1) as consts:
            ident = consts.tile([P, P], BF16, tag="ident")
            nc.sync.dma_start(ident, ident_d[:, :])
            sel2 = consts.tile([65, P], BF16, tag="sel2")
            nc.sync.dma_start(sel2, sel2_d[:, :])
            # denominator staging rows 0/64; rows 1-63 stay 1.0 (never
            # written) so ln->exp of them is finite where sel2 rows are zero
            dnP = consts.tile([65, S], F32, tag="dnP")
            nc.gpsimd.memset(dnP, 1.0)
            trimask = consts.tile([P, P], F32, tag="trimask")
            nc.sync.dma_start(trimask, trimask_d[:, :])
            # x transposed: [d-part, d-chunk, t]
            xT = consts.tile([P, NDC, S], BF16, tag="xT")
            # attention output (raw, then normalized in place), transposed:
            # [pair-hk part, pair, s]
            OcatT = consts.tile([P, NPAIR, S], BF16, tag="OcatT")

            # ---- P0: build xT via PE transposes ----
            with (
                tc.tile_pool(name="p0", bufs=3) as p0,
                tc.tile_pool(name="ps0", bufs=2, space="PSUM") as ps0,
            ):
                for st in range(NST):
                    xin = p0.tile([P, D], BF16, tag="xin")
                    nc.sync.dma_start(xin, x_d[st * P : (st + 1) * P, :])
                    for dc in range(NDC):
                        pt = ps0.tile([P, P], BF16, tag="tr")
                        nc.tensor.transpose(
                            pt, xin[:, dc * P : (dc + 1) * P], ident
                        )
                        nc.vector.tensor_copy(
                            out=xT[:, dc, st * P : (st + 1) * P], in_=pt
                        )

            # ---- P1+P2: per head-pair projections + attention ----
            with (
                tc.tile_pool(name="pw", bufs=2) as pw,
                tc.tile_pool(name="pqk", bufs=2) as pqk,
                tc.tile_pool(name="pvt", bufs=2) as pvt,
                tc.tile_pool(name="pvp", bufs=2) as pvp,
                tc.tile_pool(name="pex", bufs=2) as pex,
                tc.tile_pool(name="pnrm", bufs=4) as pnrm,
                tc.tile_pool(name="psM", bufs=2, space="PSUM") as psM,
                tc.tile_pool(name="psS", bufs=2, space="PSUM") as psS,
                tc.tile_pool(name="psO", bufs=1, space="PSUM") as psO,
            ):
                def qkv_units(p):
                    """Projection work for pair p as a list of closures, so
                    it can be interleaved into the previous pair's attention
                    loop (keeps the PE densely fed with full-array matmuls
                    while the scalar engine works through the exps)."""
                    wq_sb = pw.tile([P, NDC, P], BF16, tag="wq")
                    wk_sb = pw.tile([P, NDC, P], BF16, tag="wk")
                    wv_sb = pw.tile([P, NDC, P], BF16, tag="wv")
                    QT = pqk.tile([P, S], BF16, tag="QT")
                    KT = pqk.tile([P, S], BF16, tag="KT")
                    # V natural layout, padded to the full 128 weight columns
                    # per head: [t-part, tt, head, (64 V | 1 ones | 63 junk)].
                    # Matmul output rows 65-127 are never read, so the pad
                    # columns need no initialization.
                    Vp = pvp.tile([P, NST, 2, P], BF16, tag="Vp")
                    units = []

                    def u_dma():
                        nc.sync.dma_start(wq_sb, wq_d[p])
                        nc.sync.dma_start(wk_sb, wk_d[p])
                        nc.sync.dma_start(wv_sb, wv_d[p])
                        nc.sync.dma_start(
                            Vp[:, :, :, 64:65],
                            ones_d[:, 0:32].rearrange(
                                "p (t two one) -> p t two one", two=2, one=1
                            ),
                        )

                    units.append(u_dma)

                    # projections as half-units (4 contraction chunks each)
                    # so one lands in every attention iteration and the PE
                    # never starves while the scalar engine works the exps
                    def u_proj_halves(w_sb, dst, s0):
                        st = {}

                        def ua():
                            ps = psM.tile([P, 512], F32, tag="mm512")
                            st["ps"] = ps
                            for dc in range(4):
                                nc.tensor.matmul(
                                    ps,
                                    w_sb[:, dc],
                                    xT[:, dc, s0 : s0 + 512],
                                    start=(dc == 0),
                                    stop=False,
                                )

                        def ub():
                            ps = st.pop("ps")
                            for dc in range(4, NDC):
                                nc.tensor.matmul(
                                    ps,
                                    w_sb[:, dc],
                                    xT[:, dc, s0 : s0 + 512],
                                    start=False,
                                    stop=(dc == NDC - 1),
                                )
                            nc.vector.tensor_copy(out=dst[:, s0 : s0 + 512], in_=ps)

                        return [ua, ub]

                    vts = {}

                    def u_vproj_halves(sc):
                        s0 = sc * 512
                        st = {}

                        def ua():
                            VT = pvt.tile([P, 512], BF16, tag="VT")
                            ps = psM.tile([P, 512], F32, tag="mm512")
                            st["ps"], st["VT"] = ps, VT
                            for dc in range(4):
                                nc.tensor.matmul(
                                    ps,
                                    wv_sb[:, dc],
                                    xT[:, dc, s0 : s0 + 512],
                                    start=(dc == 0),
                                    stop=False,
                                )

                        def ub():
                            ps, VT = st.pop("ps"), st.pop("VT")
                            for dc in range(4, NDC):
                                nc.tensor.matmul(
                                    ps,
                                    wv_sb[:, dc],
                                    xT[:, dc, s0 : s0 + 512],
                                    start=False,
                                    stop=(dc == NDC - 1),
                                )
                            nc.vector.tensor_copy(out=VT, in_=ps)
                            vts[sc] = VT

                        return [ua, ub]

                    def u_vtrans(sc):
                        VT = vts.pop(sc)
                        for tq in range(4):
                            tt = sc * 4 + tq
                            ptv = psM.tile([P, 1024], BF16, tag="mm512")
                            nc.tensor.transpose(
                                ptv[:, 0:P], VT[:, tq * P : (tq + 1) * P], ident
                            )
                            for h in (0, 1):
                                nc.vector.tensor_copy(
                                    out=Vp[:, tt, h, 0:64],
                                    in_=ptv[:, 64 * h : 64 * h + 64],
                                )

                    for sc in range(NS):
                        units.extend(u_proj_halves(wq_sb, QT, sc * 512))
                        units.extend(u_proj_halves(wk_sb, KT, sc * 512))
                        units.extend(u_vproj_halves(sc))
                        if sc >= 1:
                            units.append(lambda sc=sc: u_vtrans(sc - 1))
                    units.append(lambda: u_vtrans(NS - 1))
                    return units, QT, KT, Vp

                from collections import deque

                pending = deque()
                units0, QT, KT, Vp = qkv_units(0)
                for u in units0:
                    u()
                for p in range(NPAIR):
                    if p + 1 < NPAIR:
                        un, QTn, KTn, Vpn = qkv_units(p + 1)
                        pending.extend(un)

                    # attention: both heads per t-tile, row-tiled scores
                    for sr in range(NS):
                        s0 = sr * 512
                        n_t = 4 * (sr + 1)
                        po0 = psO.tile([P, 512], F32, tag="po0", name="po0")
                        po1 = psO.tile([P, 512], F32, tag="po1", name="po1")
                        ets = {}

                        def emit_attv(tt):
                            et = ets.pop(tt)
                            nc.tensor.matmul(
                                po0,
                                Vp[:, tt, 0],
                                et[:, 0],
                                start=(tt == 0),
                                stop=(tt == n_t - 1),
                            )
                            nc.tensor.matmul(
                                po1,
                                Vp[:, tt, 1],
                                et[:, 1],
                                start=(tt == 0),
                                stop=(tt == n_t - 1),
                            )

                        for tt in range(n_t):
                            pss = psS.tile([P, 2, 512], F32, tag="s", name="s")
                            for i in (0, 1):
                                nc.tensor.matmul(
                                    pss[:, i],
                                    KT[64 * i : 64 * i + 64, tt * P : (tt + 1) * P],
                                    QT[64 * i : 64 * i + 64, s0 : s0 + 512],
                                    start=True,
                                    stop=True,
                                )
                            j = tt - 4 * sr
                            if j >= 0:
                                # triangle on the diagonal 128-block only;
                                # the fully-masked prefix columns are zeroed
                                # on GpSimd after the exp, off this chain
                                nc.vector.tensor_tensor(
                                    pss[:, :, P * j : P * (j + 1)],
                                    pss[:, :, P * j : P * (j + 1)],
                                    trimask.rearrange(
                                        "p (one k) -> p one k", one=1
                                    ).broadcast_to([P, 2, P]),
                                    mybir.AluOpType.add,
                                )
                            et = pex.tile([P, 2, 512], BF16, tag="e", name="e")
                            nc.scalar.activation(
                                out=et,
                                in_=pss,
                                func=mybir.ActivationFunctionType.Exp,
                                scale=float(HD**-0.5),
                            )
                            if j >= 1:
                                nc.gpsimd.memset(et[:, :, 0 : P * j], 0.0)
                            ets[tt] = et
                            if tt >= 1:
                                emit_attv(tt - 1)
                            if pending:
                                pending.popleft()()
                        emit_attv(n_t - 1)

                        # raw attention outputs; normalization is applied
                        # in place before the output projection reads them
                        nc.vector.tensor_copy(
                            out=OcatT[0:64, p, s0 : s0 + 512], in_=po0[0:64, :]
                        )
                        nc.vector.tensor_copy(
                            out=dnP[0:1, s0 : s0 + 512], in_=po0[64:65, :]
                        )
                        nc.vector.tensor_copy(
                            out=OcatT[64:128, p, s0 : s0 + 512], in_=po1[0:64, :]
                        )
                        nc.vector.tensor_copy(
                            out=dnP[64:65, s0 : s0 + 512], in_=po1[64:65, :]
                        )
                        # 1/den = exp(-ln(den)) on the scalar engine (the DVE
                        # reciprocal is ~6.5 ns/elem; ACT is ~1.1)
                        lnt = pnrm.tile([65, 512], F32, tag="lnt")
                        nc.scalar.activation(
                            out=lnt,
                            in_=dnP[0:65, s0 : s0 + 512],
                            func=mybir.ActivationFunctionType.Ln,
                        )
                        rcr = pnrm.tile([65, 512], BF16, tag="rcr")
                        nc.scalar.activation(
                            out=rcr,
                            in_=lnt,
                            func=mybir.ActivationFunctionType.Exp,
                            scale=-1.0,
                        )

                        def u_norm(p=p, s0=s0, rcr=rcr):
                            # deferred so the broadcast matmul's ACT-chain
                            # dependency never stalls the PE at s-run tails
                            rb = psM.tile([P, 512], F32, tag="mm512")
                            nc.tensor.matmul(rb, sel2, rcr, start=True, stop=True)
                            rbs = pnrm.tile([P, 512], BF16, tag="rbs")
                            nc.vector.tensor_copy(out=rbs, in_=rb)
                            nc.gpsimd.tensor_tensor(
                                OcatT[0:64, p, s0 : s0 + 512],
                                OcatT[0:64, p, s0 : s0 + 512],
                                rbs[0:64, :],
                                mybir.AluOpType.mult,
                            )
                            nc.gpsimd.tensor_tensor(
                                OcatT[64:128, p, s0 : s0 + 512],
                                OcatT[64:128, p, s0 : s0 + 512],
                                rbs[64:128, :],
                                mybir.AluOpType.mult,
                            )

                        pending.append(u_norm)
                    if p + 1 < NPAIR:
                        QT, KT, Vp = QTn, KTn, Vpn
                while pending:
                    pending.popleft()()

            # ---- P3: output projection (row-parallel partial) ----
            with (
                tc.tile_pool(name="p3", bufs=3) as p3,
                tc.tile_pool(name="p3w", bufs=1) as p3w,
                tc.tile_pool(name="ps3", bufs=4, space="PSUM") as ps3,
            ):
                wp_sb = p3w.tile([P, NPAIR, D], BF16, tag="wp")
                for p in range(NPAIR):
                    nc.sync.dma_start(wp_sb[:, p, :], wp_d[p])
                for st in range(NST):
                    for dc2 in range(2):
                        ps = ps3.tile([P, 512], F32, tag="y")
                        for p in range(NPAIR):
                            nc.tensor.matmul(
                                ps,
                                OcatT[:, p, st * P : (st + 1) * P],
                                wp_sb[:, p, dc2 * 512 : (dc2 + 1) * 512],
                                start=(p == 0),
                                stop=(p == NPAIR - 1),
                            )
                        yt = p3.tile([P, 512], F32, tag="yt")
                        nc.scalar.copy(out=yt, in_=ps)
                        nc.sync.dma_start(
                            y_d[st * P : (st + 1) * P, dc2 * 512 : (dc2 + 1) * 512],
                            yt,
                        )

    fix_extra_waits(nc)
    return nc


_NC = None


def _get_nc():
    global _NC
    if _NC is None:
        _NC = build_nc()
    return _NC


def _prep_core_inputs(x, Wq, Wk, Wv, Wp, core):
    b, hg = core // 2, core % 2
    hsl = slice(hg * 8, hg * 8 + 8)

    def prep_w(W):
        # [8, D, HD] -> [pair, dp, dc, (hip k)]
        a = W[hsl].reshape(NPAIR, 2, NDC, P, HD)
        return np.ascontiguousarray(
            a.transpose(0, 3, 2, 1, 4).reshape(NPAIR, P, NDC, P)
        ).astype(NPBF)

    trimask = np.where(
        np.arange(P)[None, :] >= np.arange(P)[:, None], 0.0, -1e30
    ).astype(np.float32)
    sel2 = np.zeros((65, P), dtype=NPBF)
    sel2[0, 0:64] = 1.0
    sel2[64, 64:128] = 1.0

    return {
        "x": np.ascontiguousarray(x[b]).astype(NPBF),
        "wq": prep_w(Wq),
        "wk": prep_w(Wk),
        "wv": prep_w(Wv),
        "wp": np.ascontiguousarray(
            Wp[hg * 512 : (hg + 1) * 512].reshape(NPAIR, P, D)
        ).astype(NPBF),
        "ident": np.eye(P, dtype=NPBF),
        "ones": np.ones((P, P), dtype=NPBF),
        "trimask": trimask,
        "sel2": sel2,
    }


def kernel(trace=False, **inputs):
    x = np.asarray(inputs["x"], dtype=np.float32)
    Wq = np.asarray(inputs["Wq"], dtype=np.float32)
    Wk = np.asarray(inputs["Wk"], dtype=np.float32)
    Wv = np.asarray(inputs["Wv"], dtype=np.float32)
    Wp = np.asarray(inputs["Wp"], dtype=np.float32)
    bp = np.asarray(inputs["bp"], dtype=np.float32)

    nc = _get_nc()
    in_maps = [_prep_core_inputs(x, Wq, Wk, Wv, Wp, c) for c in range(8)]
    res = run_bass_kernel_spmd(nc, in_maps, core_ids=list(range(8)), trace=trace)

    out = np.empty((B, S, D), dtype=np.float32)
    for b in range(B):
        out[b] = res.results[2 * b]["y"] + res.results[2 * b + 1]["y"] + bp
    if trace:
        return out, res
    return out
